# revision 9
# baseline (speedup 1.0000x reference)
"""Multi-head attention (softmax over the QUERY axis) on 8 TRN2 NeuronCores.

Problem shapes: Q [T=1024, B=8, D=256]; per-head projections Wq/Wk/Wv
[H=8, E=512, D=256]; Wo [D=256, H*E=4096]. Data-parallel over batch B.

Algebraic restructuring (exact): since o_h = attn_h @ v_h and
v_h = x@Wv_h^T + bv_h, associativity gives

    out = sum_h attn_h @ (x @ M_h^T + c_h) + bo,
    M_h = Wo_h @ Wv_h  (D x D, host-precomputed),  c_h = bv_h @ Wo_h^T.

This removes the V projection, the E-wide attn@V matmul and the output
projection (per-head MACs 1611M -> ~1142M).

fp8 (e4m3) DoubleRow is used for the two T^2 matmuls only:
  scores:  lg[s,t] = kT8^T-pairs x qT8      (q/k projected in bf16,
                                             cast to fp8 with scale aq)
  AV:      out^T[d,t] += Pn8-pairs x R8
with the low-error decomposition exp(lg) = 1 + R:
  R8 = fp8(exp(lg) - 1)  (3x less quantization error than fp8(exp)),
  Pn = (x@M^T + c) * ap/l   with l[s] = sum_t exp  (softmax denom, from
  the Exp activation's accum_out), and the rank-1 term
  u[d] = sum_s Pn[s,d] from a bf16 copy Pnb via N=1 matmuls against ones.

The head loop is software-pipelined: head h+1's q/k projection matmuls
are emitted between head h's scores and AV so the PE never idles while
the exp -> R -> rr -> Pn chain drains on ACT/DVE/GpSimd (PE-idle gaps
>3.4us re-throttle the HAM clock gate to half rate).
"""

import sys

sys.path.insert(0, "/opt/trn_rl_repo")

from contextlib import ExitStack

import ml_dtypes
import numpy as np

import concourse.bass as bass
import concourse.tile as tile
from concourse.tile import add_dep_helper
from concourse import bacc, bass_utils, mybir

T, B, D, H, E = 1024, 8, 256, 8, 512
N_CORES = 8
AQ = 16.0       # fp8 scale on each of qT/kT (logit psum = AQ^2 * s * qk)
AP = 8192.0     # fp8 scale on Pn

F32 = mybir.dt.float32
BF16 = mybir.dt.bfloat16
F8 = mybir.dt.float8e4
AF = mybir.ActivationFunctionType
ALU = mybir.AluOpType
DR = mybir.MatmulPerfMode.DoubleRow


def _bcast(ap_row, parts):
    """Partition-broadcast a [1, n] DRAM AP to [parts, n] (step-0 partition)."""
    return bass.AP(
        tensor=ap_row.tensor,
        offset=ap_row.offset,
        ap=[[0, parts], list(ap_row.ap[-1])],
    )


def build_nc(t=T, d=D, h=H, e=E):
    """Build the per-core SPMD program. Returns a compiled Bacc."""
    TC = t // 512   # t chunks (512-wide psum free dim)
    SB = t // 128   # s blocks
    EB = e // 128   # e blocks
    DC = d // 128   # d chunks (contraction for projections)
    DB = d // 128   # d blocks of the transposed output

    sc = 1.0 / (AQ * AQ)

    nc = bacc.Bacc("TRN2", target_bir_lowering=False, debug=False)

    qt_d = nc.dram_tensor("qt", [128, DC, t], BF16, kind="ExternalInput").ap()
    wqt_d = nc.dram_tensor("wqt", [h, 128, DC, e], BF16, kind="ExternalInput").ap()
    wkt_d = nc.dram_tensor("wkt", [h, 128, DC, e], BF16, kind="ExternalInput").ap()
    mt_d = nc.dram_tensor("mt", [h, 128, DC, d], BF16, kind="ExternalInput").ap()
    bq_d = nc.dram_tensor("bqs", [128, h, EB], F32, kind="ExternalInput").ap()
    bk_d = nc.dram_tensor("bks", [128, h, EB], F32, kind="ExternalInput").ap()
    cs_d = nc.dram_tensor("cs", [h, d], F32, kind="ExternalInput").ap()
    boc_d = nc.dram_tensor("boc", [128, DB], F32, kind="ExternalInput").ap()
    out_d = nc.dram_tensor("out", [d, t], F32, kind="ExternalOutput").ap()

    with tile.TileContext(nc) as tc, ExitStack() as ctx:
        consts = ctx.enter_context(tc.tile_pool(name="consts", bufs=1))
        wpool = ctx.enter_context(tc.tile_pool(name="wpool", bufs=2))
        hpool = ctx.enter_context(tc.tile_pool(name="hpool", bufs=2))
        qkpool = ctx.enter_context(tc.tile_pool(name="qkpool", bufs=2))
        epool = ctx.enter_context(tc.tile_pool(name="epool", bufs=3))
        spool = ctx.enter_context(tc.tile_pool(name="spool", bufs=2))
        at_pool = ctx.enter_context(tc.tile_pool(name="at_pool", bufs=3, space="PSUM"))
        mm_pool = ctx.enter_context(tc.tile_pool(name="mm_pool", bufs=4, space="PSUM"))

        # ---- persistent loads -------------------------------------------
        qt_sb = consts.tile([128, DC, t], BF16)
        nc.sync.dma_start(out=qt_sb[:, 0, :], in_=qt_d[:, 0, :])
        bq_sb = consts.tile([128, h, EB], F32)
        nc.sync.dma_start(out=bq_sb, in_=bq_d)
        bk_sb = consts.tile([128, h, EB], F32)
        nc.sync.dma_start(out=bk_sb, in_=bk_d)
        boc_sb = consts.tile([128, DB], F32)
        nc.sync.dma_start(out=boc_sb, in_=boc_d)
        out_acc = consts.tile([128, DB, t], F32)
        uacc = consts.tile([128, DB], F32)
        ones_bf = consts.tile([128, 1], BF16)
        nc.vector.memset(ones_bf, 1.0)
        out_r = out_d.rearrange("(db p) t -> p db t", p=128)

        # ---- PE warm-up: dummy matmuls during the initial DMA wait ------
        scratch = consts.tile([128, 640], BF16)
        nc.vector.memset(scratch, 0.0)
        ps_w = mm_pool.tile([128, 512], F32, tag="mm")
        for _ in range(6):
            nc.tensor.matmul(
                ps_w, scratch[:, :128], scratch[:, 128:640], start=True, stop=True
            )

        wq_sb = [None] * h
        wk_sb = [None] * h
        qT8 = [None] * h
        kT8 = [None] * h

        def load_qk_weights(hh):
            wq_sb[hh] = wpool.tile([128, DC, e], BF16, tag="wq", name="wq_sb")
            for dc in range(DC):
                nc.sync.dma_start(out=wq_sb[hh][:, dc, :], in_=wqt_d[hh, :, dc, :])
            wk_sb[hh] = wpool.tile([128, DC, e], BF16, tag="wk", name="wk_sb")
            for dc in range(DC):
                nc.sync.dma_start(out=wk_sb[hh][:, dc, :], in_=wkt_d[hh, :, dc, :])

        def qk_proj(hh):
            """q/k projections (bf16 matmul), cast to fp8 [e, t] tiles."""
            qT8[hh] = qkpool.tile([128, EB, t], F8, tag="qT", name="qT8")
            kT8[hh] = qkpool.tile([128, EB, t], F8, tag="kT", name="kT8")
            first_mm = None
            for eb in range(EB):
                for tch in range(TC):
                    tsl = slice(tch * 512, (tch + 1) * 512)
                    ps_q = mm_pool.tile([128, 512], F32, tag="mm")
                    for dc in range(DC):
                        mm = nc.tensor.matmul(
                            ps_q,
                            wq_sb[hh][:, dc, eb * 128 : (eb + 1) * 128],
                            qt_sb[:, dc, tsl],
                            start=(dc == 0),
                            stop=(dc == DC - 1),
                        )
                        if first_mm is None:
                            first_mm = mm
                    nc.vector.tensor_scalar_add(
                        qT8[hh][:, eb, tsl], ps_q, bq_sb[:, hh, eb : eb + 1]
                    )
            for eb in range(EB):
                for tch in range(TC):
                    tsl = slice(tch * 512, (tch + 1) * 512)
                    ps_k = mm_pool.tile([128, 512], F32, tag="mm")
                    for dc in range(DC):
                        nc.tensor.matmul(
                            ps_k,
                            wk_sb[hh][:, dc, eb * 128 : (eb + 1) * 128],
                            qt_sb[:, dc, tsl],
                            start=(dc == 0),
                            stop=(dc == DC - 1),
                        )
                    nc.scalar.activation(
                        kT8[hh][:, eb, tsl],
                        ps_k,
                        AF.Identity,
                        bias=bk_sb[:, hh, eb : eb + 1],
                    )
            return first_mm

        # head 0 prologue
        load_qk_weights(0)
        nc.sync.dma_start(out=qt_sb[:, 1, :], in_=qt_d[:, 1, :])
        first_mm0 = qk_proj(0)

        for hh in range(h):
            # per-head bulk loads (mt/c for this head, w for next head)
            gated = []
            mt_sb = wpool.tile([128, DC, d], BF16, tag="mt")
            gated.append(nc.sync.dma_start(out=mt_sb, in_=mt_d[hh]))
            c_bc = wpool.tile([128, d], F32, tag="c")
            gated.append(
                nc.gpsimd.dma_start(out=c_bc, in_=_bcast(cs_d[hh][None, :], 128))
            )
            if hh == 0:
                for g in gated:
                    add_dep_helper(
                        g.ins, first_mm0.ins, reason="defer bulk load past cold start"
                    )
            if hh + 1 < h:
                load_qk_weights(hh + 1)

            # ---- scores (fp8 DR) -> exp (+accum l) ----------------------
            R8 = hpool.tile([128, SB, t], F8)
            lsum2 = spool.tile([128, SB, TC], F32)
            ets = []
            for sb in range(SB):
                ssl = slice(sb * 128, (sb + 1) * 128)
                et = epool.tile([128, t], F32)
                for tch in range(TC):
                    tsl = slice(tch * 512, (tch + 1) * 512)
                    at = at_pool.tile([128, 512], F32, tag="at")
                    for i in range(EB // 2):
                        nc.tensor.matmul(
                            at,
                            kT8[hh][:, 2 * i : 2 * i + 2, ssl],
                            qT8[hh][:, 2 * i : 2 * i + 2, tsl],
                            start=(i == 0),
                            stop=(i == EB // 2 - 1),
                            perf_mode=DR,
                        )
                    nc.scalar.activation(
                        et[:, tsl],
                        at,
                        AF.Exp,
                        scale=sc,
                        accum_out=lsum2[:, sb, tch : tch + 1],
                    )
                ets.append(et)

            # ---- P projection (bf16): P32 = x @ M^T + c -----------------
            P32 = hpool.tile([128, SB, d], F32)
            for sb in range(SB):
                ssl = slice(sb * 128, (sb + 1) * 128)
                pp = mm_pool.tile([128, 512], F32, tag="mm")
                for dc in range(DC):
                    nc.tensor.matmul(
                        pp[:, :d],
                        qt_sb[:, dc, ssl],
                        mt_sb[:, dc, :],
                        start=(dc == 0),
                        stop=(dc == DC - 1),
                    )
                nc.vector.tensor_add(P32[:, sb, :], pp[:, :d], c_bc)

            # ---- PE filler: next head's q/k projections -----------------
            if hh + 1 < h:
                qk_proj(hh + 1)

            # ---- R8 = et - 1 (one wide DVE op per s-block) --------------
            for sb in range(SB):
                nc.vector.tensor_scalar(
                    R8[:, sb, :], ets[sb], 1.0, None, op0=ALU.subtract
                )

            # ---- softmax denominators: rr2 = AP / l ---------------------
            ls = spool.tile([128, SB], F32)
            lsS = spool.tile([128, SB], F32)
            rr2 = spool.tile([128, SB], F32)
            nc.vector.tensor_add(ls, lsum2[:, :, 0], lsum2[:, :, 1])
            nc.vector.tensor_scalar_mul(lsS, ls, 1.0 / AP)
            nc.vector.reciprocal(rr2, lsS)

            # ---- Pn8 (fp8 for AV) and Pnb (bf16 for u) on GpSimd --------
            Pn8 = hpool.tile([128, SB, d], F8)
            Pnb = hpool.tile([128, SB, d], BF16)
            for sb in range(SB):
                nc.gpsimd.tensor_scalar_mul(
                    Pn8[:, sb, :], P32[:, sb, :], rr2[:, sb : sb + 1]
                )
                nc.gpsimd.tensor_scalar_mul(
                    Pnb[:, sb, :], P32[:, sb, :], rr2[:, sb : sb + 1]
                )

            # ---- AV (fp8 DR): out^T[d,t] += Pn8^T-pairs x R8 ------------
            for dt in range(DB):
                dsl = slice(dt * 128, (dt + 1) * 128)
                for tch in range(TC):
                    tsl = slice(tch * 512, (tch + 1) * 512)
                    ot = at_pool.tile([128, 512], F32, tag="at")
                    for i in range(SB // 2):
                        nc.tensor.matmul(
                            ot,
                            Pn8[:, 2 * i : 2 * i + 2, dsl],
                            R8[:, 2 * i : 2 * i + 2, tsl],
                            start=(i == 0),
                            stop=(i == SB // 2 - 1),
                            perf_mode=DR,
                        )
                    if hh == 0:
                        nc.scalar.activation(out_acc[:, dt, tsl], ot, AF.Copy)
                    else:
                        nc.vector.tensor_add(
                            out_acc[:, dt, tsl], out_acc[:, dt, tsl], ot
                        )

            # ---- rank-1 term u[d] = sum_s Pnb[s,d] (bf16 N=1 matmuls) ---
            for dt in range(DB):
                dsl = slice(dt * 128, (dt + 1) * 128)
                up = mm_pool.tile([128, 512], F32, tag="mm")
                for sb in range(SB):
                    nc.tensor.matmul(
                        up[:, :1],
                        Pnb[:, sb, dsl],
                        ones_bf,
                        start=(sb == 0),
                        stop=(sb == SB - 1),
                    )
                if hh == 0:
                    nc.scalar.activation(uacc[:, dt : dt + 1], up[:, :1], AF.Copy)
                else:
                    nc.vector.tensor_add(
                        uacc[:, dt : dt + 1], uacc[:, dt : dt + 1], up[:, :1]
                    )

        # ---- final: out = (out_acc + uacc + AP*bo) / AP, store ----------
        bvec = spool.tile([128, DB], F32)
        nc.vector.tensor_add(bvec, uacc, boc_sb)
        for dt in range(DB):
            nc.vector.tensor_scalar(
                out_acc[:, dt, :],
                out_acc[:, dt, :],
                bvec[:, dt : dt + 1],
                1.0 / AP,
                op0=ALU.add,
                op1=ALU.mult,
            )
            nc.sync.dma_start(out=out_r[:, dt, :], in_=out_acc[:, dt, :])

    nc.compile()
    return nc


_NC_CACHE = {}


def _get_nc(shape_key):
    if shape_key not in _NC_CACHE:
        _NC_CACHE[shape_key] = build_nc(*shape_key)
    return _NC_CACHE[shape_key]


def _pmajor(a, last):
    """[..., C*128, last] -> [..., 128, C, last] partition-major layout."""
    lead = a.shape[:-2]
    c = a.shape[-2] // 128
    return np.ascontiguousarray(
        a.reshape(*lead, c, 128, last).swapaxes(-3, -2)
    )


def _prep_inputs(Q, Wq, bq, Wk, bk, Wv, bv, Wo, bo):
    t, b, d = Q.shape
    h, e, _ = Wq.shape
    s = np.float32(1.0 / np.sqrt(e))
    rs_aq = np.float32(np.sqrt(s) * AQ)
    bf = ml_dtypes.bfloat16
    Q = np.asarray(Q, np.float32)
    Wq = np.asarray(Wq, np.float32)
    Wk = np.asarray(Wk, np.float32)
    Wv = np.asarray(Wv, np.float32)
    Wo = np.asarray(Wo, np.float32)
    bv = np.asarray(bv, np.float32)
    bo = np.asarray(bo, np.float32)
    # [B, 128, DC, T] partition-major x^T per batch
    qt_all = _pmajor(Q.transpose(1, 2, 0).astype(bf), t)
    wqt = _pmajor((Wq.transpose(0, 2, 1) * rs_aq).astype(bf), e)
    wkt = _pmajor((Wk.transpose(0, 2, 1) * rs_aq).astype(bf), e)
    # M_h = Wo_h @ Wv_h [D, D]; mt stores M_h^T partition-major over d'
    Wo_heads = Wo.reshape(d, h, e)
    mts = np.stack([(Wo_heads[:, hh, :] @ Wv[hh]).T for hh in range(h)])
    mt = _pmajor(mts.astype(bf), d)
    cs = np.stack([bv[hh] @ Wo_heads[:, hh, :].T for hh in range(h)])
    shared = {
        "wqt": wqt,
        "wkt": wkt,
        "mt": mt,
        "bqs": np.ascontiguousarray(
            (np.asarray(bq, np.float32) * rs_aq).reshape(h, -1, 128).transpose(2, 0, 1)
        ),
        "bks": np.ascontiguousarray(
            (np.asarray(bk, np.float32) * rs_aq).reshape(h, -1, 128).transpose(2, 0, 1)
        ),
        "cs": np.ascontiguousarray(cs.astype(np.float32)),
        "boc": np.ascontiguousarray((bo * AP).reshape(-1, 128).T.astype(np.float32)),
    }
    in_maps = [
        {"qt": np.ascontiguousarray(qt_all[bb]), **shared} for bb in range(b)
    ]
    return in_maps, (t, d, h, e)


def kernel(Q, Wq, bq, Wk, bk, Wv, bv, Wo, bo, _trace=False):
    in_maps, (t, d, h, e) = _prep_inputs(Q, Wq, bq, Wk, bk, Wv, bv, Wo, bo)
    nc = _get_nc((t, d, h, e))
    res = bass_utils.run_bass_kernel_spmd(
        nc, in_maps, core_ids=list(range(len(in_maps))), trace=_trace
    )
    # per-core output is out^T [D, T]; transpose back and stack over batch
    out = np.stack(
        [res.results[bb]["out"].T for bb in range(len(in_maps))], axis=1
    )
    if _trace:
        kernel.last_results = res
    return np.ascontiguousarray(out.astype(np.float32))


# revision 10
# speedup vs baseline: 3.4161x; 3.4161x over previous
"""Multi-head attention (softmax over the QUERY axis) on 8 TRN2 NeuronCores.

Problem shapes: Q [T=1024, B=8, D=256]; per-head projections Wq/Wk/Wv
[H=8, E=512, D=256]; Wo [D=256, H*E=4096]. Data-parallel over batch B.

Algebraic restructuring (exact): since o_h = attn_h @ v_h and
v_h = x@Wv_h^T + bv_h, associativity gives

    out = sum_h attn_h @ (x @ M_h^T + c_h) + bo,
    M_h = Wo_h @ Wv_h  (D x D, host-precomputed),  c_h = bv_h @ Wo_h^T.

This removes the V projection, the E-wide attn@V matmul and the output
projection (per-head MACs 1611M -> ~1142M).

fp8 (e4m3) DoubleRow is used for the two T^2 matmuls only:
  scores:  lg[s,t] = kT8^T-pairs x qT8      (q/k projected in bf16,
                                             cast to fp8 with scale aq)
  AV:      out^T[d,t] += Pn8-pairs x R8
with the low-error decomposition exp(lg) = 1 + R:
  R8 = fp8(exp(lg) - 1)  (3x less quantization error than fp8(exp)),
  Pn = (x@M^T + c) * ap/l   with l[s] = sum_t exp  (softmax denom, from
  the Exp activation's accum_out), and the rank-1 term
  u[d] = sum_s Pn[s,d] from a bf16 copy Pnb (ScalarE, rr via the
  per-partition activation scale) via N=1 matmuls against ones.

The head loop is software-pipelined: head h+1's q/k projection matmuls
are emitted between head h's scores and AV so the PE never idles while
the exp -> R -> rr -> Pn chain drains on ACT/DVE/GpSimd (PE-idle gaps
>3.4us re-throttle the HAM clock gate to half rate).
"""

import sys

sys.path.insert(0, "/opt/trn_rl_repo")

from contextlib import ExitStack

import ml_dtypes
import numpy as np

import concourse.bass as bass
import concourse.tile as tile
from concourse.tile import add_dep_helper
from concourse import bacc, bass_utils, mybir

T, B, D, H, E = 1024, 8, 256, 8, 512
N_CORES = 8
AQ = 16.0       # fp8 scale on each of qT/kT (logit psum = AQ^2 * s * qk)
AP = 8192.0     # fp8 scale on Pn

F32 = mybir.dt.float32
BF16 = mybir.dt.bfloat16
F8 = mybir.dt.float8e4
AF = mybir.ActivationFunctionType
ALU = mybir.AluOpType
DR = mybir.MatmulPerfMode.DoubleRow


def _bcast(ap_row, parts):
    """Partition-broadcast a [1, n] DRAM AP to [parts, n] (step-0 partition)."""
    return bass.AP(
        tensor=ap_row.tensor,
        offset=ap_row.offset,
        ap=[[0, parts], list(ap_row.ap[-1])],
    )


def build_nc(t=T, d=D, h=H, e=E):
    """Build the per-core SPMD program. Returns a compiled Bacc."""
    TC = t // 512   # t chunks (512-wide psum free dim)
    SB = t // 128   # s blocks
    EB = e // 128   # e blocks
    DC = d // 128   # d chunks (contraction for projections)
    DB = d // 128   # d blocks of the transposed output

    sc = 1.0 / (AQ * AQ)

    nc = bacc.Bacc("TRN2", target_bir_lowering=False, debug=False)

    qt_d = nc.dram_tensor("qt", [128, DC, t], BF16, kind="ExternalInput").ap()
    wqt_d = nc.dram_tensor("wqt", [h, 128, DC, e], BF16, kind="ExternalInput").ap()
    wkt_d = nc.dram_tensor("wkt", [h, 128, DC, e], BF16, kind="ExternalInput").ap()
    mt_d = nc.dram_tensor("mt", [h, 128, DC, d], BF16, kind="ExternalInput").ap()
    bq_d = nc.dram_tensor("bqs", [128, h, EB], F32, kind="ExternalInput").ap()
    bk_d = nc.dram_tensor("bks", [128, h, EB], F32, kind="ExternalInput").ap()
    cs_d = nc.dram_tensor("cs", [h, d], F32, kind="ExternalInput").ap()
    boc_d = nc.dram_tensor("boc", [128, DB], F32, kind="ExternalInput").ap()
    out_d = nc.dram_tensor("out", [d, t], F32, kind="ExternalOutput").ap()

    with tile.TileContext(nc) as tc, ExitStack() as ctx:
        consts = ctx.enter_context(tc.tile_pool(name="consts", bufs=1))
        wpool = ctx.enter_context(tc.tile_pool(name="wpool", bufs=2))
        hpool = ctx.enter_context(tc.tile_pool(name="hpool", bufs=2))
        qkpool = ctx.enter_context(tc.tile_pool(name="qkpool", bufs=2))
        epool = ctx.enter_context(tc.tile_pool(name="epool", bufs=3))
        spool = ctx.enter_context(tc.tile_pool(name="spool", bufs=2))
        at_pool = ctx.enter_context(tc.tile_pool(name="at_pool", bufs=3, space="PSUM"))
        mm_pool = ctx.enter_context(tc.tile_pool(name="mm_pool", bufs=4, space="PSUM"))

        # ---- persistent loads -------------------------------------------
        qt_sb = consts.tile([128, DC, t], BF16)
        nc.sync.dma_start(out=qt_sb[:, 0, :], in_=qt_d[:, 0, :])
        bq_sb = consts.tile([128, h, EB], F32)
        nc.sync.dma_start(out=bq_sb, in_=bq_d)
        bk_sb = consts.tile([128, h, EB], F32)
        nc.sync.dma_start(out=bk_sb, in_=bk_d)
        boc_sb = consts.tile([128, DB], F32)
        nc.sync.dma_start(out=boc_sb, in_=boc_d)
        out_acc = consts.tile([128, DB, t], F32)
        uacc = consts.tile([128, DB], F32)
        ones_bf = consts.tile([128, 1], BF16)
        nc.vector.memset(ones_bf, 1.0)
        out_r = out_d.rearrange("(db p) t -> p db t", p=128)

        # ---- PE warm-up: dummy matmuls during the initial DMA wait ------
        scratch = consts.tile([128, 640], BF16)
        nc.vector.memset(scratch, 0.0)
        ps_w = mm_pool.tile([128, 512], F32, tag="mm")
        for _ in range(6):
            nc.tensor.matmul(
                ps_w, scratch[:, :128], scratch[:, 128:640], start=True, stop=True
            )

        wq_sb = [None] * h
        wk_sb = [None] * h
        qT8 = [None] * h
        kT8 = [None] * h

        def load_qk_weights(hh):
            wq_sb[hh] = wpool.tile([128, DC, e], BF16, tag="wq", name="wq_sb")
            for dc in range(DC):
                nc.sync.dma_start(out=wq_sb[hh][:, dc, :], in_=wqt_d[hh, :, dc, :])
            wk_sb[hh] = wpool.tile([128, DC, e], BF16, tag="wk", name="wk_sb")
            for dc in range(DC):
                nc.sync.dma_start(out=wk_sb[hh][:, dc, :], in_=wkt_d[hh, :, dc, :])

        def qk_proj(hh):
            """q/k projections (bf16 matmul), cast to fp8 [e, t] tiles."""
            qT8[hh] = qkpool.tile([128, EB, t], F8, tag="qT", name="qT8")
            kT8[hh] = qkpool.tile([128, EB, t], F8, tag="kT", name="kT8")
            first_mm = None
            for eb in range(EB):
                for tch in range(TC):
                    tsl = slice(tch * 512, (tch + 1) * 512)
                    ps_q = mm_pool.tile([128, 512], F32, tag="mm")
                    for dc in range(DC):
                        mm = nc.tensor.matmul(
                            ps_q,
                            wq_sb[hh][:, dc, eb * 128 : (eb + 1) * 128],
                            qt_sb[:, dc, tsl],
                            start=(dc == 0),
                            stop=(dc == DC - 1),
                        )
                        if first_mm is None:
                            first_mm = mm
                    nc.vector.tensor_scalar_add(
                        qT8[hh][:, eb, tsl], ps_q, bq_sb[:, hh, eb : eb + 1]
                    )
            for eb in range(EB):
                for tch in range(TC):
                    tsl = slice(tch * 512, (tch + 1) * 512)
                    ps_k = mm_pool.tile([128, 512], F32, tag="mm")
                    for dc in range(DC):
                        nc.tensor.matmul(
                            ps_k,
                            wk_sb[hh][:, dc, eb * 128 : (eb + 1) * 128],
                            qt_sb[:, dc, tsl],
                            start=(dc == 0),
                            stop=(dc == DC - 1),
                        )
                    nc.scalar.activation(
                        kT8[hh][:, eb, tsl],
                        ps_k,
                        AF.Identity,
                        bias=bk_sb[:, hh, eb : eb + 1],
                    )
            return first_mm

        # head 0 prologue
        load_qk_weights(0)
        nc.sync.dma_start(out=qt_sb[:, 1, :], in_=qt_d[:, 1, :])
        first_mm0 = qk_proj(0)

        for hh in range(h):
            # per-head bulk loads (mt/c for this head, w for next head)
            gated = []
            mt_sb = wpool.tile([128, DC, d], BF16, tag="mt")
            gated.append(nc.sync.dma_start(out=mt_sb, in_=mt_d[hh]))
            c_bc = wpool.tile([128, d], F32, tag="c")
            gated.append(
                nc.gpsimd.dma_start(out=c_bc, in_=_bcast(cs_d[hh][None, :], 128))
            )
            if hh == 0:
                for g in gated:
                    add_dep_helper(
                        g.ins, first_mm0.ins, reason="defer bulk load past cold start"
                    )
            if hh + 1 < h:
                load_qk_weights(hh + 1)

            # ---- scores (fp8 DR) -> exp (+accum l) ----------------------
            R8 = hpool.tile([128, SB, t], F8)
            lsum2 = spool.tile([128, SB, TC], F32)
            ets = []
            for sb in range(SB):
                ssl = slice(sb * 128, (sb + 1) * 128)
                et = epool.tile([128, t], BF16)
                for tch in range(TC):
                    tsl = slice(tch * 512, (tch + 1) * 512)
                    at = at_pool.tile([128, 512], F32, tag="at")
                    for i in range(EB // 2):
                        nc.tensor.matmul(
                            at,
                            kT8[hh][:, 2 * i : 2 * i + 2, ssl],
                            qT8[hh][:, 2 * i : 2 * i + 2, tsl],
                            start=(i == 0),
                            stop=(i == EB // 2 - 1),
                            perf_mode=DR,
                        )
                    nc.scalar.activation(
                        et[:, tsl],
                        at,
                        AF.Exp,
                        scale=sc,
                        accum_out=lsum2[:, sb, tch : tch + 1],
                    )
                ets.append(et)

            # ---- P projection (bf16): P32 = x @ M^T + c -----------------
            P32 = hpool.tile([128, SB, d], F32)
            for sb in range(SB):
                ssl = slice(sb * 128, (sb + 1) * 128)
                pp = mm_pool.tile([128, 512], F32, tag="mm")
                for dc in range(DC):
                    nc.tensor.matmul(
                        pp[:, :d],
                        qt_sb[:, dc, ssl],
                        mt_sb[:, dc, :],
                        start=(dc == 0),
                        stop=(dc == DC - 1),
                    )
                nc.vector.tensor_add(P32[:, sb, :], pp[:, :d], c_bc)

            # ---- PE filler: next head's q/k projections -----------------
            if hh + 1 < h:
                qk_proj(hh + 1)

            # ---- R8 = et - 1 (one wide DVE op per s-block) --------------
            for sb in range(SB):
                nc.vector.tensor_scalar(
                    R8[:, sb, :], ets[sb], 1.0, None, op0=ALU.subtract
                )

            # ---- softmax denominators: rr2 = AP / l ---------------------
            ls = spool.tile([128, SB], F32)
            lsS = spool.tile([128, SB], F32)
            rr2 = spool.tile([128, SB], F32)
            nc.vector.tensor_add(ls, lsum2[:, :, 0], lsum2[:, :, 1])
            nc.vector.tensor_scalar_mul(lsS, ls, 1.0 / AP)
            nc.vector.reciprocal(rr2, lsS)

            # ---- Pn8 (fp8 for AV) and Pnb (bf16 for u) on GpSimd --------
            Pn8 = hpool.tile([128, SB, d], F8)
            Pnb = hpool.tile([128, SB, d], BF16)
            for sb in range(SB):
                nc.vector.tensor_scalar_mul(
                    Pn8[:, sb, :], P32[:, sb, :], rr2[:, sb : sb + 1]
                )
                nc.scalar.mul(Pnb[:, sb, :], P32[:, sb, :], rr2[:, sb : sb + 1])

            # ---- AV (fp8 DR): out^T[d,t] += Pn8^T-pairs x R8 ------------
            for dt in range(DB):
                dsl = slice(dt * 128, (dt + 1) * 128)
                for tch in range(TC):
                    tsl = slice(tch * 512, (tch + 1) * 512)
                    ot = at_pool.tile([128, 512], F32, tag="at")
                    for i in range(SB // 2):
                        nc.tensor.matmul(
                            ot,
                            Pn8[:, 2 * i : 2 * i + 2, dsl],
                            R8[:, 2 * i : 2 * i + 2, tsl],
                            start=(i == 0),
                            stop=(i == SB // 2 - 1),
                            perf_mode=DR,
                        )
                    if hh == 0:
                        nc.scalar.activation(out_acc[:, dt, tsl], ot, AF.Copy)
                    else:
                        nc.vector.tensor_add(
                            out_acc[:, dt, tsl], out_acc[:, dt, tsl], ot
                        )

            # ---- rank-1 term u[d] = sum_s Pnb[s,d] (bf16 N=1 matmuls) ---
            for dt in range(DB):
                dsl = slice(dt * 128, (dt + 1) * 128)
                up = mm_pool.tile([128, 512], F32, tag="mm")
                for sb in range(SB):
                    nc.tensor.matmul(
                        up[:, :1],
                        Pnb[:, sb, dsl],
                        ones_bf,
                        start=(sb == 0),
                        stop=(sb == SB - 1),
                    )
                if hh == 0:
                    nc.scalar.activation(uacc[:, dt : dt + 1], up[:, :1], AF.Copy)
                else:
                    nc.vector.tensor_add(
                        uacc[:, dt : dt + 1], uacc[:, dt : dt + 1], up[:, :1]
                    )

        # ---- final: out = (out_acc + uacc + AP*bo) / AP, store ----------
        bvec = spool.tile([128, DB], F32)
        nc.vector.tensor_add(bvec, uacc, boc_sb)
        for dt in range(DB):
            nc.vector.tensor_scalar(
                out_acc[:, dt, :],
                out_acc[:, dt, :],
                bvec[:, dt : dt + 1],
                1.0 / AP,
                op0=ALU.add,
                op1=ALU.mult,
            )
            nc.sync.dma_start(out=out_r[:, dt, :], in_=out_acc[:, dt, :])

    nc.compile()
    return nc


_NC_CACHE = {}


def _get_nc(shape_key):
    if shape_key not in _NC_CACHE:
        _NC_CACHE[shape_key] = build_nc(*shape_key)
    return _NC_CACHE[shape_key]


def _pmajor(a, last):
    """[..., C*128, last] -> [..., 128, C, last] partition-major layout."""
    lead = a.shape[:-2]
    c = a.shape[-2] // 128
    return np.ascontiguousarray(
        a.reshape(*lead, c, 128, last).swapaxes(-3, -2)
    )


def _prep_inputs(Q, Wq, bq, Wk, bk, Wv, bv, Wo, bo):
    t, b, d = Q.shape
    h, e, _ = Wq.shape
    s = np.float32(1.0 / np.sqrt(e))
    rs_aq = np.float32(np.sqrt(s) * AQ)
    bf = ml_dtypes.bfloat16
    Q = np.asarray(Q, np.float32)
    Wq = np.asarray(Wq, np.float32)
    Wk = np.asarray(Wk, np.float32)
    Wv = np.asarray(Wv, np.float32)
    Wo = np.asarray(Wo, np.float32)
    bv = np.asarray(bv, np.float32)
    bo = np.asarray(bo, np.float32)
    # [B, 128, DC, T] partition-major x^T per batch
    qt_all = _pmajor(Q.transpose(1, 2, 0).astype(bf), t)
    wqt = _pmajor((Wq.transpose(0, 2, 1) * rs_aq).astype(bf), e)
    wkt = _pmajor((Wk.transpose(0, 2, 1) * rs_aq).astype(bf), e)
    # M_h = Wo_h @ Wv_h [D, D]; mt stores M_h^T partition-major over d'
    Wo_heads = Wo.reshape(d, h, e)
    mts = np.stack([(Wo_heads[:, hh, :] @ Wv[hh]).T for hh in range(h)])
    mt = _pmajor(mts.astype(bf), d)
    cs = np.stack([bv[hh] @ Wo_heads[:, hh, :].T for hh in range(h)])
    shared = {
        "wqt": wqt,
        "wkt": wkt,
        "mt": mt,
        "bqs": np.ascontiguousarray(
            (np.asarray(bq, np.float32) * rs_aq).reshape(h, -1, 128).transpose(2, 0, 1)
        ),
        "bks": np.ascontiguousarray(
            (np.asarray(bk, np.float32) * rs_aq).reshape(h, -1, 128).transpose(2, 0, 1)
        ),
        "cs": np.ascontiguousarray(cs.astype(np.float32)),
        "boc": np.ascontiguousarray((bo * AP).reshape(-1, 128).T.astype(np.float32)),
    }
    in_maps = [
        {"qt": np.ascontiguousarray(qt_all[bb]), **shared} for bb in range(b)
    ]
    return in_maps, (t, d, h, e)


def kernel(Q, Wq, bq, Wk, bk, Wv, bv, Wo, bo, _trace=False):
    in_maps, (t, d, h, e) = _prep_inputs(Q, Wq, bq, Wk, bk, Wv, bv, Wo, bo)
    nc = _get_nc((t, d, h, e))
    res = bass_utils.run_bass_kernel_spmd(
        nc, in_maps, core_ids=list(range(len(in_maps))), trace=_trace
    )
    # per-core output is out^T [D, T]; transpose back and stack over batch
    out = np.stack(
        [res.results[bb]["out"].T for bb in range(len(in_maps))], axis=1
    )
    if _trace:
        kernel.last_results = res
    return np.ascontiguousarray(out.astype(np.float32))


# revision 11
# speedup vs baseline: 3.4331x; 1.0050x over previous
"""Multi-head attention (softmax over the QUERY axis) on 8 TRN2 NeuronCores.

Problem shapes: Q [T=1024, B=8, D=256]; per-head projections Wq/Wk/Wv
[H=8, E=512, D=256]; Wo [D=256, H*E=4096]. Data-parallel over batch B.

Algebraic restructuring (exact): since o_h = attn_h @ v_h and
v_h = x@Wv_h^T + bv_h, associativity gives

    out = sum_h attn_h @ (x @ M_h^T + c_h) + bo,
    M_h = Wo_h @ Wv_h  (D x D, host-precomputed),  c_h = bv_h @ Wo_h^T.

This removes the V projection, the E-wide attn@V matmul and the output
projection (per-head MACs 1611M -> ~1142M).

fp8 (e4m3) DoubleRow is used for the two T^2 matmuls only:
  scores:  lg[s,t] = kT8^T-pairs x qT8      (q/k projected in bf16,
                                             cast to fp8 with scale aq)
  AV:      out^T[d,t] += Pn8-pairs x R8
with the low-error decomposition exp(lg) = 1 + R:
  R8 = fp8(exp(lg) - 1)  (3x less quantization error than fp8(exp)),
  Pn = (x@M^T + c) * ap/l   with l[s] = sum_t exp  (softmax denom, from
  the Exp activation's accum_out), and the rank-1 term
  u[d] = sum_s Pn[s,d] from a bf16 copy Pnb (ScalarE, rr via the
  per-partition activation scale) via N=1 matmuls against ones.

The head loop is software-pipelined: head h+1's q/k projection matmuls
are emitted between head h's scores and AV so the PE never idles while
the exp -> R -> rr -> Pn chain drains on ACT/DVE/GpSimd (PE-idle gaps
>3.4us re-throttle the HAM clock gate to half rate).
"""

import sys

sys.path.insert(0, "/opt/trn_rl_repo")

from contextlib import ExitStack

import ml_dtypes
import numpy as np

import concourse.bass as bass
import concourse.tile as tile
from concourse.tile import add_dep_helper
from concourse import bacc, bass_utils, mybir

T, B, D, H, E = 1024, 8, 256, 8, 512
N_CORES = 8
AQ = 16.0       # fp8 scale on each of qT/kT (logit psum = AQ^2 * s * qk)
AP = 8192.0     # fp8 scale on Pn

F32 = mybir.dt.float32
BF16 = mybir.dt.bfloat16
F8 = mybir.dt.float8e4
AF = mybir.ActivationFunctionType
ALU = mybir.AluOpType
DR = mybir.MatmulPerfMode.DoubleRow


def _bcast(ap_row, parts):
    """Partition-broadcast a [1, n] DRAM AP to [parts, n] (step-0 partition)."""
    return bass.AP(
        tensor=ap_row.tensor,
        offset=ap_row.offset,
        ap=[[0, parts], list(ap_row.ap[-1])],
    )


def build_nc(t=T, d=D, h=H, e=E):
    """Build the per-core SPMD program. Returns a compiled Bacc."""
    TC = t // 512   # t chunks (512-wide psum free dim)
    SB = t // 128   # s blocks
    EB = e // 128   # e blocks
    DC = d // 128   # d chunks (contraction for projections)
    DB = d // 128   # d blocks of the transposed output

    sc = 1.0 / (AQ * AQ)

    nc = bacc.Bacc("TRN2", target_bir_lowering=False, debug=False)

    qt_d = nc.dram_tensor("qt", [128, DC, t], BF16, kind="ExternalInput").ap()
    wqt_d = nc.dram_tensor("wqt", [h, 128, DC, e], BF16, kind="ExternalInput").ap()
    wkt_d = nc.dram_tensor("wkt", [h, 128, DC, e], BF16, kind="ExternalInput").ap()
    mt_d = nc.dram_tensor("mt", [h, 128, DC, d], BF16, kind="ExternalInput").ap()
    bq_d = nc.dram_tensor("bqs", [128, h, EB], F32, kind="ExternalInput").ap()
    bk_d = nc.dram_tensor("bks", [128, h, EB], F32, kind="ExternalInput").ap()
    cs_d = nc.dram_tensor("cs", [h, d], F32, kind="ExternalInput").ap()
    boc_d = nc.dram_tensor("boc", [128, DB], F32, kind="ExternalInput").ap()
    out_d = nc.dram_tensor("out", [d, t], F32, kind="ExternalOutput").ap()

    with tile.TileContext(nc) as tc, ExitStack() as ctx:
        consts = ctx.enter_context(tc.tile_pool(name="consts", bufs=1))
        wpool = ctx.enter_context(tc.tile_pool(name="wpool", bufs=2))
        hpool = ctx.enter_context(tc.tile_pool(name="hpool", bufs=2))
        qkpool = ctx.enter_context(tc.tile_pool(name="qkpool", bufs=2))
        epool = ctx.enter_context(tc.tile_pool(name="epool", bufs=3))
        spool = ctx.enter_context(tc.tile_pool(name="spool", bufs=2))
        at_pool = ctx.enter_context(tc.tile_pool(name="at_pool", bufs=3, space="PSUM"))
        mm_pool = ctx.enter_context(tc.tile_pool(name="mm_pool", bufs=4, space="PSUM"))

        # ---- persistent loads -------------------------------------------
        qt_sb = consts.tile([128, DC, t], BF16)
        nc.sync.dma_start(out=qt_sb[:, 0, :], in_=qt_d[:, 0, :])
        bq_sb = consts.tile([128, h, EB], F32)
        nc.sync.dma_start(out=bq_sb, in_=bq_d)
        bk_sb = consts.tile([128, h, EB], F32)
        nc.sync.dma_start(out=bk_sb, in_=bk_d)
        boc_sb = consts.tile([128, DB], F32)
        nc.sync.dma_start(out=boc_sb, in_=boc_d)
        out_acc = consts.tile([128, DB, t], F32)
        uacc = consts.tile([128, DB], F32)
        ones_bf = consts.tile([128, 1], BF16)
        nc.vector.memset(ones_bf, 1.0)
        out_r = out_d.rearrange("(db p) t -> p db t", p=128)

        # ---- PE warm-up: dummy matmuls during the initial DMA wait ------
        scratch = consts.tile([128, 640], BF16)
        nc.vector.memset(scratch, 0.0)
        ps_w = mm_pool.tile([128, 512], F32, tag="mm")
        for _ in range(6):
            nc.tensor.matmul(
                ps_w, scratch[:, :128], scratch[:, 128:640], start=True, stop=True
            )

        wq_sb = [None] * h
        wk_sb = [None] * h
        qT8 = [None] * h
        kT8 = [None] * h

        def load_qk_weights(hh):
            wq_sb[hh] = wpool.tile([128, DC, e], BF16, tag="wq", name="wq_sb")
            for dc in range(DC):
                nc.sync.dma_start(out=wq_sb[hh][:, dc, :], in_=wqt_d[hh, :, dc, :])
            wk_sb[hh] = wpool.tile([128, DC, e], BF16, tag="wk", name="wk_sb")
            for dc in range(DC):
                nc.sync.dma_start(out=wk_sb[hh][:, dc, :], in_=wkt_d[hh, :, dc, :])

        def qk_proj(hh):
            """q/k projections (bf16 matmul), cast to fp8 [e, t] tiles."""
            qT8[hh] = qkpool.tile([128, EB, t], F8, tag="qT", name="qT8")
            kT8[hh] = qkpool.tile([128, EB, t], F8, tag="kT", name="kT8")
            first_mm = None
            for eb in range(EB):
                for tch in range(TC):
                    tsl = slice(tch * 512, (tch + 1) * 512)
                    ps_q = mm_pool.tile([128, 512], F32, tag="mm")
                    for dc in range(DC):
                        mm = nc.tensor.matmul(
                            ps_q,
                            wq_sb[hh][:, dc, eb * 128 : (eb + 1) * 128],
                            qt_sb[:, dc, tsl],
                            start=(dc == 0),
                            stop=(dc == DC - 1),
                        )
                        if first_mm is None:
                            first_mm = mm
                    nc.vector.tensor_scalar_add(
                        qT8[hh][:, eb, tsl], ps_q, bq_sb[:, hh, eb : eb + 1]
                    )
            for eb in range(EB):
                for tch in range(TC):
                    tsl = slice(tch * 512, (tch + 1) * 512)
                    ps_k = mm_pool.tile([128, 512], F32, tag="mm")
                    for dc in range(DC):
                        nc.tensor.matmul(
                            ps_k,
                            wk_sb[hh][:, dc, eb * 128 : (eb + 1) * 128],
                            qt_sb[:, dc, tsl],
                            start=(dc == 0),
                            stop=(dc == DC - 1),
                        )
                    nc.scalar.activation(
                        kT8[hh][:, eb, tsl],
                        ps_k,
                        AF.Identity,
                        bias=bk_sb[:, hh, eb : eb + 1],
                    )
            return first_mm

        # head 0 prologue
        load_qk_weights(0)
        nc.sync.dma_start(out=qt_sb[:, 1, :], in_=qt_d[:, 1, :])
        first_mm0 = qk_proj(0)

        for hh in range(h):
            # per-head bulk loads (mt/c for this head, w for next head)
            gated = []
            mt_sb = wpool.tile([128, DC, d], BF16, tag="mt")
            gated.append(nc.sync.dma_start(out=mt_sb, in_=mt_d[hh]))
            c_bc = wpool.tile([128, d], F32, tag="c")
            gated.append(
                nc.gpsimd.dma_start(out=c_bc, in_=_bcast(cs_d[hh][None, :], 128))
            )
            if hh == 0:
                for g in gated:
                    add_dep_helper(
                        g.ins, first_mm0.ins, reason="defer bulk load past cold start"
                    )
            if hh + 1 < h:
                load_qk_weights(hh + 1)

            # ---- scores (fp8 DR) -> exp (+accum l) ----------------------
            R8 = hpool.tile([128, SB, t], F8)
            lsum = spool.tile([128, SB], F32)
            ets = []
            for sb in range(SB):
                ssl = slice(sb * 128, (sb + 1) * 128)
                et = epool.tile([128, t], BF16)
                for tch in range(TC):
                    tsl = slice(tch * 512, (tch + 1) * 512)
                    at = at_pool.tile([128, 512], F32, tag="at")
                    for i in range(EB // 2):
                        nc.tensor.matmul(
                            at,
                            kT8[hh][:, 2 * i : 2 * i + 2, ssl],
                            qT8[hh][:, 2 * i : 2 * i + 2, tsl],
                            start=(i == 0),
                            stop=(i == EB // 2 - 1),
                            perf_mode=DR,
                        )
                    nc.scalar.activation(et[:, tsl], at, AF.Exp, scale=sc)
                ets.append(et)

            # ---- P projection (bf16): P32 = x @ M^T + c -----------------
            P32 = hpool.tile([128, SB, d], F32)
            for sb in range(SB):
                ssl = slice(sb * 128, (sb + 1) * 128)
                pp = mm_pool.tile([128, 512], F32, tag="mm")
                for dc in range(DC):
                    nc.tensor.matmul(
                        pp[:, :d],
                        qt_sb[:, dc, ssl],
                        mt_sb[:, dc, :],
                        start=(dc == 0),
                        stop=(dc == DC - 1),
                    )
                nc.vector.tensor_add(P32[:, sb, :], pp[:, :d], c_bc)

            # ---- PE filler: next head's q/k projections -----------------
            if hh + 1 < h:
                qk_proj(hh + 1)

            # ---- R8 = et - 1 (one wide DVE op per s-block) --------------
            for sb in range(SB):
                nc.vector.tensor_scalar(
                    R8[:, sb, :],
                    ets[sb],
                    1.0,
                    0.0,
                    op0=ALU.subtract,
                    op1=ALU.add,
                    accum_out=lsum[:, sb : sb + 1],
                )

            # ---- softmax denominators: rr2 = AP / l ---------------------
            lsS = spool.tile([128, SB], F32)
            rr2 = spool.tile([128, SB], F32)
            nc.vector.tensor_scalar(
                lsS, lsum, float(t), 1.0 / AP, op0=ALU.add, op1=ALU.mult
            )
            nc.vector.reciprocal(rr2, lsS)

            # ---- Pn8 (fp8 for AV) and Pnb (bf16 for u) on GpSimd --------
            Pn8 = hpool.tile([128, SB, d], F8)
            Pnb = hpool.tile([128, SB, d], BF16)
            for sb in range(SB):
                nc.vector.tensor_scalar_mul(
                    Pn8[:, sb, :], P32[:, sb, :], rr2[:, sb : sb + 1]
                )
                nc.scalar.mul(Pnb[:, sb, :], P32[:, sb, :], rr2[:, sb : sb + 1])

            # ---- AV (fp8 DR): out^T[d,t] += Pn8^T-pairs x R8 ------------
            for dt in range(DB):
                dsl = slice(dt * 128, (dt + 1) * 128)
                for tch in range(TC):
                    tsl = slice(tch * 512, (tch + 1) * 512)
                    ot = at_pool.tile([128, 512], F32, tag="at")
                    for i in range(SB // 2):
                        nc.tensor.matmul(
                            ot,
                            Pn8[:, 2 * i : 2 * i + 2, dsl],
                            R8[:, 2 * i : 2 * i + 2, tsl],
                            start=(i == 0),
                            stop=(i == SB // 2 - 1),
                            perf_mode=DR,
                        )
                    if hh == 0:
                        nc.scalar.activation(out_acc[:, dt, tsl], ot, AF.Copy)
                    else:
                        nc.vector.tensor_add(
                            out_acc[:, dt, tsl], out_acc[:, dt, tsl], ot
                        )

            # ---- rank-1 term u[d] = sum_s Pnb[s,d] (bf16 N=1 matmuls) ---
            for dt in range(DB):
                dsl = slice(dt * 128, (dt + 1) * 128)
                up = mm_pool.tile([128, 512], F32, tag="mm")
                for sb in range(SB):
                    nc.tensor.matmul(
                        up[:, :1],
                        Pnb[:, sb, dsl],
                        ones_bf,
                        start=(sb == 0),
                        stop=(sb == SB - 1),
                    )
                if hh == 0:
                    nc.scalar.activation(uacc[:, dt : dt + 1], up[:, :1], AF.Copy)
                else:
                    nc.vector.tensor_add(
                        uacc[:, dt : dt + 1], uacc[:, dt : dt + 1], up[:, :1]
                    )

        # ---- final: out = (out_acc + uacc + AP*bo) / AP, store ----------
        bvec = spool.tile([128, DB], F32)
        nc.vector.tensor_add(bvec, uacc, boc_sb)
        for dt in range(DB):
            nc.vector.tensor_scalar(
                out_acc[:, dt, :],
                out_acc[:, dt, :],
                bvec[:, dt : dt + 1],
                1.0 / AP,
                op0=ALU.add,
                op1=ALU.mult,
            )
            nc.sync.dma_start(out=out_r[:, dt, :], in_=out_acc[:, dt, :])

    nc.compile()
    return nc


_NC_CACHE = {}


def _get_nc(shape_key):
    if shape_key not in _NC_CACHE:
        _NC_CACHE[shape_key] = build_nc(*shape_key)
    return _NC_CACHE[shape_key]


def _pmajor(a, last):
    """[..., C*128, last] -> [..., 128, C, last] partition-major layout."""
    lead = a.shape[:-2]
    c = a.shape[-2] // 128
    return np.ascontiguousarray(
        a.reshape(*lead, c, 128, last).swapaxes(-3, -2)
    )


def _prep_inputs(Q, Wq, bq, Wk, bk, Wv, bv, Wo, bo):
    t, b, d = Q.shape
    h, e, _ = Wq.shape
    s = np.float32(1.0 / np.sqrt(e))
    rs_aq = np.float32(np.sqrt(s) * AQ)
    bf = ml_dtypes.bfloat16
    Q = np.asarray(Q, np.float32)
    Wq = np.asarray(Wq, np.float32)
    Wk = np.asarray(Wk, np.float32)
    Wv = np.asarray(Wv, np.float32)
    Wo = np.asarray(Wo, np.float32)
    bv = np.asarray(bv, np.float32)
    bo = np.asarray(bo, np.float32)
    # [B, 128, DC, T] partition-major x^T per batch
    qt_all = _pmajor(Q.transpose(1, 2, 0).astype(bf), t)
    wqt = _pmajor((Wq.transpose(0, 2, 1) * rs_aq).astype(bf), e)
    wkt = _pmajor((Wk.transpose(0, 2, 1) * rs_aq).astype(bf), e)
    # M_h = Wo_h @ Wv_h [D, D]; mt stores M_h^T partition-major over d'
    Wo_heads = Wo.reshape(d, h, e)
    mts = np.stack([(Wo_heads[:, hh, :] @ Wv[hh]).T for hh in range(h)])
    mt = _pmajor(mts.astype(bf), d)
    cs = np.stack([bv[hh] @ Wo_heads[:, hh, :].T for hh in range(h)])
    shared = {
        "wqt": wqt,
        "wkt": wkt,
        "mt": mt,
        "bqs": np.ascontiguousarray(
            (np.asarray(bq, np.float32) * rs_aq).reshape(h, -1, 128).transpose(2, 0, 1)
        ),
        "bks": np.ascontiguousarray(
            (np.asarray(bk, np.float32) * rs_aq).reshape(h, -1, 128).transpose(2, 0, 1)
        ),
        "cs": np.ascontiguousarray(cs.astype(np.float32)),
        "boc": np.ascontiguousarray((bo * AP).reshape(-1, 128).T.astype(np.float32)),
    }
    in_maps = [
        {"qt": np.ascontiguousarray(qt_all[bb]), **shared} for bb in range(b)
    ]
    return in_maps, (t, d, h, e)


def kernel(Q, Wq, bq, Wk, bk, Wv, bv, Wo, bo, _trace=False):
    in_maps, (t, d, h, e) = _prep_inputs(Q, Wq, bq, Wk, bk, Wv, bv, Wo, bo)
    nc = _get_nc((t, d, h, e))
    res = bass_utils.run_bass_kernel_spmd(
        nc, in_maps, core_ids=list(range(len(in_maps))), trace=_trace
    )
    # per-core output is out^T [D, T]; transpose back and stack over batch
    out = np.stack(
        [res.results[bb]["out"].T for bb in range(len(in_maps))], axis=1
    )
    if _trace:
        kernel.last_results = res
    return np.ascontiguousarray(out.astype(np.float32))


# revision 12
# speedup vs baseline: 3.7211x; 1.0839x over previous
"""Multi-head attention (softmax over the QUERY axis) on 8 TRN2 NeuronCores.

Problem shapes: Q [T=1024, B=8, D=256]; per-head projections Wq/Wk/Wv
[H=8, E=512, D=256]; Wo [D=256, H*E=4096]. Data-parallel over batch B.

Algebraic restructuring (exact): since o_h = attn_h @ v_h and
v_h = x@Wv_h^T + bv_h, associativity gives

    out = sum_h attn_h @ (x @ M_h^T + c_h) + bo,
    M_h = Wo_h @ Wv_h  (D x D, host-precomputed),  c_h = bv_h @ Wo_h^T.

This removes the V projection, the E-wide attn@V matmul and the output
projection (per-head MACs 1611M -> ~1142M).

fp8 (e4m3) DoubleRow is used for the two T^2 matmuls only:
  scores:  lg[s,t] = kT8^T-pairs x qT8      (q/k projected in bf16,
                                             cast to fp8 with scale aq)
  AV:      out^T[d,t] += Pn8-pairs x R8
with the low-error decomposition exp(lg) = 1 + R:
  R8 = fp8(exp(lg) - 1)  (3x less quantization error than fp8(exp)),
  Pn = (x@M^T + c) * ap/l   with l[s] = sum_t exp  (softmax denom, from
  the Exp activation's accum_out), and the rank-1 term
  u[d] = sum_s Pn[s,d] from a bf16 copy Pnb (ScalarE, rr via the
  per-partition activation scale) via N=1 matmuls against ones.

The head loop is software-pipelined: head h+1's q/k projection matmuls
are emitted between head h's scores and AV so the PE never idles while
the exp -> R -> rr -> Pn chain drains on ACT/DVE/GpSimd (PE-idle gaps
>3.4us re-throttle the HAM clock gate to half rate).
"""

import sys

sys.path.insert(0, "/opt/trn_rl_repo")

from contextlib import ExitStack

import ml_dtypes
import numpy as np

import concourse.bass as bass
import concourse.tile as tile
from concourse.tile import add_dep_helper
from concourse import bacc, bass_utils, mybir

T, B, D, H, E = 1024, 8, 256, 8, 512
N_CORES = 8
AQ = 16.0       # fp8 scale on each of qT/kT (logit psum = AQ^2 * s * qk)
AP = 8192.0     # fp8 scale on Pn

F32 = mybir.dt.float32
BF16 = mybir.dt.bfloat16
F8 = mybir.dt.float8e4
AF = mybir.ActivationFunctionType
ALU = mybir.AluOpType
DR = mybir.MatmulPerfMode.DoubleRow


def _bcast(ap_row, parts):
    """Partition-broadcast a [1, n] DRAM AP to [parts, n] (step-0 partition)."""
    return bass.AP(
        tensor=ap_row.tensor,
        offset=ap_row.offset,
        ap=[[0, parts], list(ap_row.ap[-1])],
    )


def build_nc(t=T, d=D, h=H, e=E):
    """Build the per-core SPMD program. Returns a compiled Bacc."""
    TC = t // 512   # t chunks (512-wide psum free dim)
    SB = t // 128   # s blocks
    EB = e // 128   # e blocks
    DC = d // 128   # d chunks (contraction for projections)
    DB = d // 128   # d blocks of the transposed output

    sc = 1.0 / (AQ * AQ)

    nc = bacc.Bacc("TRN2", target_bir_lowering=False, debug=False)

    qt_d = nc.dram_tensor("qt", [128, DC, t], BF16, kind="ExternalInput").ap()
    wqt_d = nc.dram_tensor("wqt", [h, 128, DC, e], BF16, kind="ExternalInput").ap()
    wkt_d = nc.dram_tensor("wkt", [h, 128, DC, e], BF16, kind="ExternalInput").ap()
    mt_d = nc.dram_tensor("mt", [h, 128, DC, d], BF16, kind="ExternalInput").ap()
    bq_d = nc.dram_tensor("bqs", [128, h, EB], F32, kind="ExternalInput").ap()
    bk_d = nc.dram_tensor("bks", [128, h, EB], F32, kind="ExternalInput").ap()
    cs_d = nc.dram_tensor("cs", [h, d], F32, kind="ExternalInput").ap()
    boc_d = nc.dram_tensor("boc", [128, DB], F32, kind="ExternalInput").ap()
    out_d = nc.dram_tensor("out", [d, t], F32, kind="ExternalOutput").ap()

    with tile.TileContext(nc) as tc, ExitStack() as ctx:
        consts = ctx.enter_context(tc.tile_pool(name="consts", bufs=1))
        wpool = ctx.enter_context(tc.tile_pool(name="wpool", bufs=2))
        hpool = ctx.enter_context(tc.tile_pool(name="hpool", bufs=2))
        qkpool = ctx.enter_context(tc.tile_pool(name="qkpool", bufs=2))
        epool = ctx.enter_context(tc.tile_pool(name="epool", bufs=3))
        spool = ctx.enter_context(tc.tile_pool(name="spool", bufs=2))
        at_pool = ctx.enter_context(tc.tile_pool(name="at_pool", bufs=3, space="PSUM"))
        mm_pool = ctx.enter_context(tc.tile_pool(name="mm_pool", bufs=4, space="PSUM"))

        # ---- persistent loads -------------------------------------------
        qt_sb = consts.tile([128, DC, t], BF16)
        nc.sync.dma_start(out=qt_sb[:, 0, :], in_=qt_d[:, 0, :])
        bq_sb = consts.tile([128, h, EB], F32)
        nc.sync.dma_start(out=bq_sb, in_=bq_d)
        bk_sb = consts.tile([128, h, EB], F32)
        nc.sync.dma_start(out=bk_sb, in_=bk_d)
        boc_sb = consts.tile([128, DB], F32)
        nc.sync.dma_start(out=boc_sb, in_=boc_d)
        out_acc = consts.tile([128, DB, t], F32)
        uacc = consts.tile([128, DB], F32)
        ones_bf = consts.tile([128, 1], BF16)
        nc.vector.memset(ones_bf, 1.0)
        out_r = out_d.rearrange("(db p) t -> p db t", p=128)

        # ---- PE warm-up: dummy matmuls during the initial DMA wait ------
        scratch = consts.tile([128, 640], BF16)
        nc.vector.memset(scratch, 0.0)
        ps_w = mm_pool.tile([128, 512], F32, tag="mm")
        for _ in range(6):
            nc.tensor.matmul(
                ps_w, scratch[:, :128], scratch[:, 128:640], start=True, stop=True
            )

        wq_sb = [None] * h
        wk_sb = [None] * h
        qT8 = [None] * h
        kT8 = [None] * h

        def load_qk_weights(hh):
            wq_sb[hh] = wpool.tile([128, DC, e], BF16, tag="wq", name="wq_sb")
            for dc in range(DC):
                nc.sync.dma_start(out=wq_sb[hh][:, dc, :], in_=wqt_d[hh, :, dc, :])
            wk_sb[hh] = wpool.tile([128, DC, e], BF16, tag="wk", name="wk_sb")
            for dc in range(DC):
                nc.sync.dma_start(out=wk_sb[hh][:, dc, :], in_=wkt_d[hh, :, dc, :])

        def qk_proj(hh):
            """q/k projections (bf16 matmul), cast to fp8 [e, t] tiles."""
            qT8[hh] = qkpool.tile([128, EB, t], F8, tag="qT", name="qT8")
            kT8[hh] = qkpool.tile([128, EB, t], F8, tag="kT", name="kT8")
            first_mm = None
            for eb in range(EB):
                for tch in range(TC):
                    tsl = slice(tch * 512, (tch + 1) * 512)
                    ps_q = mm_pool.tile([128, 512], F32, tag="mm")
                    for dc in range(DC):
                        mm = nc.tensor.matmul(
                            ps_q,
                            wq_sb[hh][:, dc, eb * 128 : (eb + 1) * 128],
                            qt_sb[:, dc, tsl],
                            start=(dc == 0),
                            stop=(dc == DC - 1),
                        )
                        if first_mm is None:
                            first_mm = mm
                    nc.vector.tensor_scalar_add(
                        qT8[hh][:, eb, tsl], ps_q, bq_sb[:, hh, eb : eb + 1]
                    )
            for eb in range(EB):
                for tch in range(TC):
                    tsl = slice(tch * 512, (tch + 1) * 512)
                    ps_k = mm_pool.tile([128, 512], F32, tag="mm")
                    for dc in range(DC):
                        nc.tensor.matmul(
                            ps_k,
                            wk_sb[hh][:, dc, eb * 128 : (eb + 1) * 128],
                            qt_sb[:, dc, tsl],
                            start=(dc == 0),
                            stop=(dc == DC - 1),
                        )
                    nc.scalar.activation(
                        kT8[hh][:, eb, tsl],
                        ps_k,
                        AF.Identity,
                        bias=bk_sb[:, hh, eb : eb + 1],
                    )
            return first_mm

        # head 0 prologue
        load_qk_weights(0)
        nc.sync.dma_start(out=qt_sb[:, 1, :], in_=qt_d[:, 1, :])
        first_mm0 = qk_proj(0)

        for hh in range(h):
            # per-head bulk loads (mt/c for this head, w for next head)
            gated = []
            mt_sb = wpool.tile([128, DC, d], BF16, tag="mt")
            gated.append(nc.sync.dma_start(out=mt_sb, in_=mt_d[hh]))
            c_bc = wpool.tile([128, d], F32, tag="c")
            gated.append(
                nc.gpsimd.dma_start(out=c_bc, in_=_bcast(cs_d[hh][None, :], 128))
            )
            if hh == 0:
                for g in gated:
                    add_dep_helper(
                        g.ins, first_mm0.ins, reason="defer bulk load past cold start"
                    )
            if hh + 1 < h:
                load_qk_weights(hh + 1)

            # ---- P projection (bf16): P32 = x @ M^T + c -----------------
            # (emitted before scores so the P-adds drain on DVE while the
            # scores matmuls run, keeping the DVE queue clear for R-subs)
            P32 = hpool.tile([128, SB, d], F32)
            for sb in range(SB):
                ssl = slice(sb * 128, (sb + 1) * 128)
                pp = mm_pool.tile([128, 512], F32, tag="mm")
                for dc in range(DC):
                    nc.tensor.matmul(
                        pp[:, :d],
                        qt_sb[:, dc, ssl],
                        mt_sb[:, dc, :],
                        start=(dc == 0),
                        stop=(dc == DC - 1),
                    )
                nc.vector.tensor_add(P32[:, sb, :], pp[:, :d], c_bc)

            # ---- scores (fp8 DR) -> exp -> R8 = exp-1 (+accum l) --------
            R8 = hpool.tile([128, SB, t], F8)
            lsum = spool.tile([128, SB], F32)
            for sb in range(SB):
                ssl = slice(sb * 128, (sb + 1) * 128)
                et = epool.tile([128, t], BF16)
                for tch in range(TC):
                    tsl = slice(tch * 512, (tch + 1) * 512)
                    at = at_pool.tile([128, 512], F32, tag="at")
                    for i in range(EB // 2):
                        nc.tensor.matmul(
                            at,
                            kT8[hh][:, 2 * i : 2 * i + 2, ssl],
                            qT8[hh][:, 2 * i : 2 * i + 2, tsl],
                            start=(i == 0),
                            stop=(i == EB // 2 - 1),
                            perf_mode=DR,
                        )
                    nc.scalar.activation(et[:, tsl], at, AF.Exp, scale=sc)
                nc.vector.tensor_scalar(
                    R8[:, sb, :],
                    et,
                    1.0,
                    0.0,
                    op0=ALU.subtract,
                    op1=ALU.add,
                    accum_out=lsum[:, sb : sb + 1],
                )

            # ---- softmax denominators: rr2 = AP / l ---------------------
            lsS = spool.tile([128, SB], F32)
            rr2 = spool.tile([128, SB], F32)
            nc.vector.tensor_scalar(
                lsS, lsum, float(t), 1.0 / AP, op0=ALU.add, op1=ALU.mult
            )
            nc.vector.reciprocal(rr2, lsS)

            # ---- Pn8 (fp8 for AV, DVE) and Pnb (bf16 for u, ScalarE) ----
            Pn8 = hpool.tile([128, SB, d], F8)
            Pnb = hpool.tile([128, SB, d], BF16)
            for sb in range(SB):
                nc.vector.tensor_scalar_mul(
                    Pn8[:, sb, :], P32[:, sb, :], rr2[:, sb : sb + 1]
                )
                nc.scalar.mul(Pnb[:, sb, :], P32[:, sb, :], rr2[:, sb : sb + 1])

            # ---- PE filler: next head's q/k projections -----------------
            # (PE runs these between the scores and AV matmuls while the
            # exp -> R -> rr -> Pn chain drains on ACT/DVE)
            if hh + 1 < h:
                qk_proj(hh + 1)

            # ---- AV (fp8 DR): out^T[d,t] += Pn8^T-pairs x R8 ------------
            for dt in range(DB):
                dsl = slice(dt * 128, (dt + 1) * 128)
                for tch in range(TC):
                    tsl = slice(tch * 512, (tch + 1) * 512)
                    ot = at_pool.tile([128, 512], F32, tag="at")
                    for i in range(SB // 2):
                        nc.tensor.matmul(
                            ot,
                            Pn8[:, 2 * i : 2 * i + 2, dsl],
                            R8[:, 2 * i : 2 * i + 2, tsl],
                            start=(i == 0),
                            stop=(i == SB // 2 - 1),
                            perf_mode=DR,
                        )
                    if hh == 0:
                        nc.scalar.activation(out_acc[:, dt, tsl], ot, AF.Copy)
                    else:
                        nc.vector.tensor_add(
                            out_acc[:, dt, tsl], out_acc[:, dt, tsl], ot
                        )

            # ---- rank-1 term u[d] = sum_s Pnb[s,d] (bf16 N=1 matmuls) ---
            for dt in range(DB):
                dsl = slice(dt * 128, (dt + 1) * 128)
                up = mm_pool.tile([128, 512], F32, tag="mm")
                for sb in range(SB):
                    nc.tensor.matmul(
                        up[:, :1],
                        Pnb[:, sb, dsl],
                        ones_bf,
                        start=(sb == 0),
                        stop=(sb == SB - 1),
                    )
                if hh == 0:
                    nc.scalar.activation(uacc[:, dt : dt + 1], up[:, :1], AF.Copy)
                else:
                    nc.vector.tensor_add(
                        uacc[:, dt : dt + 1], uacc[:, dt : dt + 1], up[:, :1]
                    )

        # ---- final: out = (out_acc + uacc + AP*bo) / AP, store ----------
        bvec = spool.tile([128, DB], F32)
        nc.vector.tensor_add(bvec, uacc, boc_sb)
        for dt in range(DB):
            nc.vector.tensor_scalar(
                out_acc[:, dt, :],
                out_acc[:, dt, :],
                bvec[:, dt : dt + 1],
                1.0 / AP,
                op0=ALU.add,
                op1=ALU.mult,
            )
            nc.sync.dma_start(out=out_r[:, dt, :], in_=out_acc[:, dt, :])

    nc.compile()
    return nc


_NC_CACHE = {}


def _get_nc(shape_key):
    if shape_key not in _NC_CACHE:
        _NC_CACHE[shape_key] = build_nc(*shape_key)
    return _NC_CACHE[shape_key]


def _pmajor(a, last):
    """[..., C*128, last] -> [..., 128, C, last] partition-major layout."""
    lead = a.shape[:-2]
    c = a.shape[-2] // 128
    return np.ascontiguousarray(
        a.reshape(*lead, c, 128, last).swapaxes(-3, -2)
    )


def _prep_inputs(Q, Wq, bq, Wk, bk, Wv, bv, Wo, bo):
    t, b, d = Q.shape
    h, e, _ = Wq.shape
    s = np.float32(1.0 / np.sqrt(e))
    rs_aq = np.float32(np.sqrt(s) * AQ)
    bf = ml_dtypes.bfloat16
    Q = np.asarray(Q, np.float32)
    Wq = np.asarray(Wq, np.float32)
    Wk = np.asarray(Wk, np.float32)
    Wv = np.asarray(Wv, np.float32)
    Wo = np.asarray(Wo, np.float32)
    bv = np.asarray(bv, np.float32)
    bo = np.asarray(bo, np.float32)
    # [B, 128, DC, T] partition-major x^T per batch
    qt_all = _pmajor(Q.transpose(1, 2, 0).astype(bf), t)
    wqt = _pmajor((Wq.transpose(0, 2, 1) * rs_aq).astype(bf), e)
    wkt = _pmajor((Wk.transpose(0, 2, 1) * rs_aq).astype(bf), e)
    # M_h = Wo_h @ Wv_h [D, D]; mt stores M_h^T partition-major over d'
    Wo_heads = Wo.reshape(d, h, e)
    mts = np.stack([(Wo_heads[:, hh, :] @ Wv[hh]).T for hh in range(h)])
    mt = _pmajor(mts.astype(bf), d)
    cs = np.stack([bv[hh] @ Wo_heads[:, hh, :].T for hh in range(h)])
    shared = {
        "wqt": wqt,
        "wkt": wkt,
        "mt": mt,
        "bqs": np.ascontiguousarray(
            (np.asarray(bq, np.float32) * rs_aq).reshape(h, -1, 128).transpose(2, 0, 1)
        ),
        "bks": np.ascontiguousarray(
            (np.asarray(bk, np.float32) * rs_aq).reshape(h, -1, 128).transpose(2, 0, 1)
        ),
        "cs": np.ascontiguousarray(cs.astype(np.float32)),
        "boc": np.ascontiguousarray((bo * AP).reshape(-1, 128).T.astype(np.float32)),
    }
    in_maps = [
        {"qt": np.ascontiguousarray(qt_all[bb]), **shared} for bb in range(b)
    ]
    return in_maps, (t, d, h, e)


def kernel(Q, Wq, bq, Wk, bk, Wv, bv, Wo, bo, _trace=False):
    in_maps, (t, d, h, e) = _prep_inputs(Q, Wq, bq, Wk, bk, Wv, bv, Wo, bo)
    nc = _get_nc((t, d, h, e))
    res = bass_utils.run_bass_kernel_spmd(
        nc, in_maps, core_ids=list(range(len(in_maps))), trace=_trace
    )
    # per-core output is out^T [D, T]; transpose back and stack over batch
    out = np.stack(
        [res.results[bb]["out"].T for bb in range(len(in_maps))], axis=1
    )
    if _trace:
        kernel.last_results = res
    return np.ascontiguousarray(out.astype(np.float32))


# revision 13
# speedup vs baseline: 3.8343x; 1.0304x over previous
"""Multi-head attention (softmax over the QUERY axis) on 8 TRN2 NeuronCores.

Problem shapes: Q [T=1024, B=8, D=256]; per-head projections Wq/Wk/Wv
[H=8, E=512, D=256]; Wo [D=256, H*E=4096]. Data-parallel over batch B.

Algebraic restructuring (exact): since o_h = attn_h @ v_h and
v_h = x@Wv_h^T + bv_h, associativity gives

    out = sum_h attn_h @ (x @ M_h^T + c_h) + bo,
    M_h = Wo_h @ Wv_h  (D x D, host-precomputed),  c_h = bv_h @ Wo_h^T.

This removes the V projection, the E-wide attn@V matmul and the output
projection (per-head MACs 1611M -> ~1142M).

fp8 (e4m3) DoubleRow is used for the two T^2 matmuls only:
  scores:  lg[s,t] = kT8^T-pairs x qT8      (q/k projected in bf16,
                                             cast to fp8 with scale aq)
  AV:      out^T[d,t] += Pn8-pairs x R8
with the low-error decomposition exp(lg) = 1 + R:
  R8 = fp8(exp(lg) - 1)  (3x less quantization error than fp8(exp)),
  Pn = (x@M^T + c) * ap/l   with l[s] = sum_t exp  (softmax denom, from
  the Exp activation's accum_out), and the rank-1 term
  u[d] = sum_s Pn[s,d] from a bf16 copy Pnb (ScalarE, rr via the
  per-partition activation scale) via N=1 matmuls against ones.

The head loop is software-pipelined: head h+1's q/k projection matmuls
are emitted between head h's scores and AV so the PE never idles while
the exp -> R -> rr -> Pn chain drains on ACT/DVE/GpSimd (PE-idle gaps
>3.4us re-throttle the HAM clock gate to half rate).
"""

import sys

sys.path.insert(0, "/opt/trn_rl_repo")

from contextlib import ExitStack

import ml_dtypes
import numpy as np

import concourse.bass as bass
import concourse.tile as tile
from concourse.tile import add_dep_helper
from concourse import bacc, bass_utils, mybir

T, B, D, H, E = 1024, 8, 256, 8, 512
N_CORES = 8
AQ = 16.0       # fp8 scale on each of qT/kT (logit psum = AQ^2 * s * qk)
AP = 8192.0     # fp8 scale on Pn

F32 = mybir.dt.float32
BF16 = mybir.dt.bfloat16
F8 = mybir.dt.float8e4
AF = mybir.ActivationFunctionType
ALU = mybir.AluOpType
DR = mybir.MatmulPerfMode.DoubleRow


def _bcast(ap_row, parts):
    """Partition-broadcast a [1, n] DRAM AP to [parts, n] (step-0 partition)."""
    return bass.AP(
        tensor=ap_row.tensor,
        offset=ap_row.offset,
        ap=[[0, parts], list(ap_row.ap[-1])],
    )


def build_nc(t=T, d=D, h=H, e=E):
    """Build the per-core SPMD program. Returns a compiled Bacc."""
    TC = t // 512   # t chunks (512-wide psum free dim)
    SB = t // 128   # s blocks
    EB = e // 128   # e blocks
    DC = d // 128   # d chunks (contraction for projections)
    DB = d // 128   # d blocks of the transposed output

    sc = 1.0 / (AQ * AQ)

    nc = bacc.Bacc("TRN2", target_bir_lowering=False, debug=False)

    qt_d = nc.dram_tensor("qt", [128, DC, t], BF16, kind="ExternalInput").ap()
    wqt_d = nc.dram_tensor("wqt", [h, 128, DC, e], BF16, kind="ExternalInput").ap()
    wkt_d = nc.dram_tensor("wkt", [h, 128, DC, e], BF16, kind="ExternalInput").ap()
    mt_d = nc.dram_tensor("mt", [h, 128, DC, d], BF16, kind="ExternalInput").ap()
    bq_d = nc.dram_tensor("bqs", [128, h, EB], F32, kind="ExternalInput").ap()
    bk_d = nc.dram_tensor("bks", [128, h, EB], F32, kind="ExternalInput").ap()
    cs_d = nc.dram_tensor("cs", [h, d], F32, kind="ExternalInput").ap()
    boc_d = nc.dram_tensor("boc", [128, DB], F32, kind="ExternalInput").ap()
    out_d = nc.dram_tensor("out", [d, t], F32, kind="ExternalOutput").ap()

    with tile.TileContext(nc) as tc, ExitStack() as ctx:
        consts = ctx.enter_context(tc.tile_pool(name="consts", bufs=1))
        wpool = ctx.enter_context(tc.tile_pool(name="wpool", bufs=2))
        hpool = ctx.enter_context(tc.tile_pool(name="hpool", bufs=2))
        qkpool = ctx.enter_context(tc.tile_pool(name="qkpool", bufs=2))
        epool = ctx.enter_context(tc.tile_pool(name="epool", bufs=3))
        spool = ctx.enter_context(tc.tile_pool(name="spool", bufs=2))
        at_pool = ctx.enter_context(tc.tile_pool(name="at_pool", bufs=3, space="PSUM"))
        mm_pool = ctx.enter_context(tc.tile_pool(name="mm_pool", bufs=5, space="PSUM"))

        # ---- persistent loads -------------------------------------------
        qt_sb = consts.tile([128, DC, t], BF16)
        nc.sync.dma_start(out=qt_sb[:, 0, :], in_=qt_d[:, 0, :])
        bq_sb = consts.tile([128, h, EB], F32)
        nc.sync.dma_start(out=bq_sb, in_=bq_d)
        bk_sb = consts.tile([128, h, EB], F32)
        nc.sync.dma_start(out=bk_sb, in_=bk_d)
        boc_sb = consts.tile([128, DB], F32)
        nc.sync.dma_start(out=boc_sb, in_=boc_d)
        out_acc = consts.tile([128, DB, t], F32)
        uacc = consts.tile([128, DB], F32)
        ones_bf = consts.tile([128, 1], BF16)
        nc.vector.memset(ones_bf, 1.0)
        out_r = out_d.rearrange("(db p) t -> p db t", p=128)

        # ---- PE warm-up: dummy matmuls during the initial DMA wait ------
        scratch = consts.tile([128, 640], BF16)
        nc.vector.memset(scratch, 0.0)
        ps_w = mm_pool.tile([128, 512], F32, tag="mm")
        for _ in range(6):
            nc.tensor.matmul(
                ps_w, scratch[:, :128], scratch[:, 128:640], start=True, stop=True
            )

        wq_sb = [None] * h
        wk_sb = [None] * h
        qT8 = [None] * h
        kT8 = [None] * h

        def load_qk_weights(hh):
            wq_sb[hh] = wpool.tile([128, DC, e], BF16, tag="wq", name="wq_sb")
            for dc in range(DC):
                nc.sync.dma_start(out=wq_sb[hh][:, dc, :], in_=wqt_d[hh, :, dc, :])
            wk_sb[hh] = wpool.tile([128, DC, e], BF16, tag="wk", name="wk_sb")
            for dc in range(DC):
                nc.sync.dma_start(out=wk_sb[hh][:, dc, :], in_=wkt_d[hh, :, dc, :])

        def q_proj(hh):
            """q projection (bf16 matmul), cast to fp8 [e, t]; bias on DVE."""
            qT8[hh] = qkpool.tile([128, EB, t], F8, tag="qT", name="qT8")
            first_mm = None
            for eb in range(EB):
                for tch in range(TC):
                    tsl = slice(tch * 512, (tch + 1) * 512)
                    ps_q = mm_pool.tile([128, 512], F32, tag="mm")
                    for dc in range(DC):
                        mm = nc.tensor.matmul(
                            ps_q,
                            wq_sb[hh][:, dc, eb * 128 : (eb + 1) * 128],
                            qt_sb[:, dc, tsl],
                            start=(dc == 0),
                            stop=(dc == DC - 1),
                        )
                        if first_mm is None:
                            first_mm = mm
                    nc.vector.tensor_scalar_add(
                        qT8[hh][:, eb, tsl], ps_q, bq_sb[:, hh, eb : eb + 1]
                    )
            return first_mm

        def k_proj(hh):
            """k projection (bf16 matmul), cast to fp8 [e, t]; bias on ACT."""
            kT8[hh] = qkpool.tile([128, EB, t], F8, tag="kT", name="kT8")
            for eb in range(EB):
                for tch in range(TC):
                    tsl = slice(tch * 512, (tch + 1) * 512)
                    ps_k = mm_pool.tile([128, 512], F32, tag="mm")
                    for dc in range(DC):
                        nc.tensor.matmul(
                            ps_k,
                            wk_sb[hh][:, dc, eb * 128 : (eb + 1) * 128],
                            qt_sb[:, dc, tsl],
                            start=(dc == 0),
                            stop=(dc == DC - 1),
                        )
                    nc.scalar.activation(
                        kT8[hh][:, eb, tsl],
                        ps_k,
                        AF.Identity,
                        bias=bk_sb[:, hh, eb : eb + 1],
                    )

        # head 0 prologue
        load_qk_weights(0)
        nc.sync.dma_start(out=qt_sb[:, 1, :], in_=qt_d[:, 1, :])
        first_mm0 = q_proj(0)
        k_proj(0)

        pnb_pending = []

        def flush_u():
            """rank-1 term u[d] = sum_s Pnb[s,d] via bf16 N=1 matmuls."""
            while pnb_pending:
                uh, Pnb = pnb_pending.pop(0)
                for dt in range(DB):
                    dsl = slice(dt * 128, (dt + 1) * 128)
                    up = mm_pool.tile([128, 512], F32, tag="mm")
                    for sb in range(SB):
                        nc.tensor.matmul(
                            up[:, :1],
                            Pnb[:, sb, dsl],
                            ones_bf,
                            start=(sb == 0),
                            stop=(sb == SB - 1),
                        )
                    if uh == 0:
                        nc.scalar.activation(
                            uacc[:, dt : dt + 1], up[:, :1], AF.Copy
                        )
                    else:
                        nc.vector.tensor_add(
                            uacc[:, dt : dt + 1], uacc[:, dt : dt + 1], up[:, :1]
                        )

        for hh in range(h):
            # per-head bulk loads (mt/c for this head, w for next head)
            gated = []
            mt_sb = wpool.tile([128, DC, d], BF16, tag="mt")
            gated.append(nc.sync.dma_start(out=mt_sb, in_=mt_d[hh]))
            c_bc = wpool.tile([128, d], F32, tag="c")
            gated.append(
                nc.gpsimd.dma_start(out=c_bc, in_=_bcast(cs_d[hh][None, :], 128))
            )
            if hh == 0:
                for g in gated:
                    add_dep_helper(
                        g.ins, first_mm0.ins, reason="defer bulk load past cold start"
                    )
            if hh + 1 < h:
                load_qk_weights(hh + 1)

            # ---- P projection (bf16): P32 = x @ M^T + c -----------------
            # (emitted before scores so the P-adds drain on DVE while the
            # scores matmuls run, keeping the DVE queue clear for R-subs)
            P32 = hpool.tile([128, SB, d], F32)
            for sb in range(SB):
                ssl = slice(sb * 128, (sb + 1) * 128)
                pp = mm_pool.tile([128, 512], F32, tag="mm")
                for dc in range(DC):
                    nc.tensor.matmul(
                        pp[:, :d],
                        qt_sb[:, dc, ssl],
                        mt_sb[:, dc, :],
                        start=(dc == 0),
                        stop=(dc == DC - 1),
                    )
                nc.vector.tensor_add(P32[:, sb, :], pp[:, :d], c_bc)

            # deferred rank-1 u matmuls of the previous head (Pnb ready)
            flush_u()

            # ---- scores (fp8 DR) -> exp -> R8 = exp-1 (+accum l) --------
            R8 = hpool.tile([128, SB, t], F8)
            lsum = spool.tile([128, SB], F32)
            for sb in range(SB):
                ssl = slice(sb * 128, (sb + 1) * 128)
                et = epool.tile([128, t], BF16)
                for tch in range(TC):
                    tsl = slice(tch * 512, (tch + 1) * 512)
                    at = at_pool.tile([128, 512], F32, tag="at")
                    for i in range(EB // 2):
                        nc.tensor.matmul(
                            at,
                            kT8[hh][:, 2 * i : 2 * i + 2, ssl],
                            qT8[hh][:, 2 * i : 2 * i + 2, tsl],
                            start=(i == 0),
                            stop=(i == EB // 2 - 1),
                            perf_mode=DR,
                        )
                    nc.scalar.activation(et[:, tsl], at, AF.Exp, scale=sc)
                nc.vector.tensor_scalar(
                    R8[:, sb, :],
                    et,
                    1.0,
                    0.0,
                    op0=ALU.subtract,
                    op1=ALU.add,
                    accum_out=lsum[:, sb : sb + 1],
                )

            # ---- softmax denominators: rr2 = AP / l ---------------------
            lsS = spool.tile([128, SB], F32)
            rr2 = spool.tile([128, SB], F32)
            nc.vector.tensor_scalar(
                lsS, lsum, float(t), 1.0 / AP, op0=ALU.add, op1=ALU.mult
            )
            nc.vector.reciprocal(rr2, lsS)

            # ---- PE filler 1: next head's q projection ------------------
            if hh + 1 < h:
                q_proj(hh + 1)

            # ---- Pn8 (fp8 for AV) on DVE (queues after the q bias-adds) -
            Pn8 = hpool.tile([128, SB, d], F8)
            for sb in range(SB):
                nc.vector.tensor_scalar_mul(
                    Pn8[:, sb, :], P32[:, sb, :], rr2[:, sb : sb + 1]
                )

            # ---- PE filler 2: next head's k projection ------------------
            if hh + 1 < h:
                k_proj(hh + 1)

            # ---- AV (fp8 DR): out^T[d,t] += Pn8^T-pairs x R8 ------------
            for dt in range(DB):
                dsl = slice(dt * 128, (dt + 1) * 128)
                for tch in range(TC):
                    tsl = slice(tch * 512, (tch + 1) * 512)
                    ot = at_pool.tile([128, 512], F32, tag="at")
                    for i in range(SB // 2):
                        nc.tensor.matmul(
                            ot,
                            Pn8[:, 2 * i : 2 * i + 2, dsl],
                            R8[:, 2 * i : 2 * i + 2, tsl],
                            start=(i == 0),
                            stop=(i == SB // 2 - 1),
                            perf_mode=DR,
                        )
                    if hh == 0:
                        nc.scalar.activation(out_acc[:, dt, tsl], ot, AF.Copy)
                    else:
                        nc.vector.tensor_add(
                            out_acc[:, dt, tsl], out_acc[:, dt, tsl], ot
                        )

            # ---- Pnb (bf16 for the rank-1 u term) on ScalarE ------------
            Pnb = hpool.tile([128, SB, d], BF16)
            for sb in range(SB):
                nc.scalar.mul(Pnb[:, sb, :], P32[:, sb, :], rr2[:, sb : sb + 1])
            pnb_pending.append((hh, Pnb))

        flush_u()

        # ---- final: out = (out_acc + uacc + AP*bo) / AP, store ----------
        bvec = spool.tile([128, DB], F32)
        nc.vector.tensor_add(bvec, uacc, boc_sb)
        for dt in range(DB):
            nc.vector.tensor_scalar(
                out_acc[:, dt, :],
                out_acc[:, dt, :],
                bvec[:, dt : dt + 1],
                1.0 / AP,
                op0=ALU.add,
                op1=ALU.mult,
            )
            nc.sync.dma_start(out=out_r[:, dt, :], in_=out_acc[:, dt, :])

    nc.compile()
    return nc


_NC_CACHE = {}


def _get_nc(shape_key):
    if shape_key not in _NC_CACHE:
        _NC_CACHE[shape_key] = build_nc(*shape_key)
    return _NC_CACHE[shape_key]


def _pmajor(a, last):
    """[..., C*128, last] -> [..., 128, C, last] partition-major layout."""
    lead = a.shape[:-2]
    c = a.shape[-2] // 128
    return np.ascontiguousarray(
        a.reshape(*lead, c, 128, last).swapaxes(-3, -2)
    )


def _prep_inputs(Q, Wq, bq, Wk, bk, Wv, bv, Wo, bo):
    t, b, d = Q.shape
    h, e, _ = Wq.shape
    s = np.float32(1.0 / np.sqrt(e))
    rs_aq = np.float32(np.sqrt(s) * AQ)
    bf = ml_dtypes.bfloat16
    Q = np.asarray(Q, np.float32)
    Wq = np.asarray(Wq, np.float32)
    Wk = np.asarray(Wk, np.float32)
    Wv = np.asarray(Wv, np.float32)
    Wo = np.asarray(Wo, np.float32)
    bv = np.asarray(bv, np.float32)
    bo = np.asarray(bo, np.float32)
    # [B, 128, DC, T] partition-major x^T per batch
    qt_all = _pmajor(Q.transpose(1, 2, 0).astype(bf), t)
    wqt = _pmajor((Wq.transpose(0, 2, 1) * rs_aq).astype(bf), e)
    wkt = _pmajor((Wk.transpose(0, 2, 1) * rs_aq).astype(bf), e)
    # M_h = Wo_h @ Wv_h [D, D]; mt stores M_h^T partition-major over d'
    Wo_heads = Wo.reshape(d, h, e)
    mts = np.stack([(Wo_heads[:, hh, :] @ Wv[hh]).T for hh in range(h)])
    mt = _pmajor(mts.astype(bf), d)
    cs = np.stack([bv[hh] @ Wo_heads[:, hh, :].T for hh in range(h)])
    shared = {
        "wqt": wqt,
        "wkt": wkt,
        "mt": mt,
        "bqs": np.ascontiguousarray(
            (np.asarray(bq, np.float32) * rs_aq).reshape(h, -1, 128).transpose(2, 0, 1)
        ),
        "bks": np.ascontiguousarray(
            (np.asarray(bk, np.float32) * rs_aq).reshape(h, -1, 128).transpose(2, 0, 1)
        ),
        "cs": np.ascontiguousarray(cs.astype(np.float32)),
        "boc": np.ascontiguousarray((bo * AP).reshape(-1, 128).T.astype(np.float32)),
    }
    in_maps = [
        {"qt": np.ascontiguousarray(qt_all[bb]), **shared} for bb in range(b)
    ]
    return in_maps, (t, d, h, e)


def kernel(Q, Wq, bq, Wk, bk, Wv, bv, Wo, bo, _trace=False):
    in_maps, (t, d, h, e) = _prep_inputs(Q, Wq, bq, Wk, bk, Wv, bv, Wo, bo)
    nc = _get_nc((t, d, h, e))
    res = bass_utils.run_bass_kernel_spmd(
        nc, in_maps, core_ids=list(range(len(in_maps))), trace=_trace
    )
    # per-core output is out^T [D, T]; transpose back and stack over batch
    out = np.stack(
        [res.results[bb]["out"].T for bb in range(len(in_maps))], axis=1
    )
    if _trace:
        kernel.last_results = res
    return np.ascontiguousarray(out.astype(np.float32))


# revision 14
# speedup vs baseline: 4.1958x; 1.0943x over previous
"""Multi-head attention (softmax over the QUERY axis) on 8 TRN2 NeuronCores.

Problem shapes: Q [T=1024, B=8, D=256]; per-head projections Wq/Wk/Wv
[H=8, E=512, D=256]; Wo [D=256, H*E=4096]. Data-parallel over batch B.

Two exact algebraic restructurings (both exploit E > D):

1. V/output side: since o_h = attn_h @ v_h and v_h = x@Wv_h^T + bv_h,
       out = sum_h attn_h @ (x @ M_h^T + c_h) + bo,
       M_h = Wo_h @ Wv_h  (D x D, host),  c_h = bv_h @ Wo_h^T.
   Removes the V projection, the E-wide attn@V matmul and the output
   projection.

2. Q/K side: q_t . k_s = x_t . (G_h x_s) with G_h = Wq_h^T @ Wk_h
   (D x D, host).  The softmax is over the QUERY axis t, so per-key
   additive terms (bq.k_s, bq.bk) cancel EXACTLY and only
   w1_h = s*Wq_h^T @ bk_h survives as a bias on the z projection:
       lg[t,s] = x_t . z_s,   z = x @ (s*G_h) + w1_h.
   Removes both the q and k projections; scores contract over D=256
   instead of E=512.

Per-head MACs: 1611M -> 672M.  fp8 (e4m3) DoubleRow runs the two T^2
matmuls (scores z8 x x8, AV Pn8 x R8); everything else is bf16.
Precision tricks: exp(lg) = 1 + R with R8 = fp8(exp - 1) (3x less
quantization error than fp8(exp)); softmax denominators l[s] = sum_t R
+ T from the R-subtract's accum_out; the rank-1 term
u[d] = sum_s Pn[s,d] from a bf16 copy Pnb via N=1 matmuls against ones
(computing u from fp8 Pn8 would sum 1024 independent quantization
errors).

The head loop is software-pipelined: head h+1's z/P projections and
head h-1's u matmuls fill the PE between head h's scores and AV so the
PE never idles while the exp -> R -> rr -> Pn chain drains on ACT/DVE
(PE-idle gaps >3.4us re-throttle the HAM clock gate to half rate).
"""

import sys

sys.path.insert(0, "/opt/trn_rl_repo")

from contextlib import ExitStack

import ml_dtypes
import numpy as np

import concourse.bass as bass
import concourse.tile as tile
from concourse.tile import add_dep_helper
from concourse import bacc, bass_utils, mybir

T, B, D, H, E = 1024, 8, 256, 8, 512
N_CORES = 8
AX = 8.0        # fp8 scale on x8 (and folded into qt host-side)
AZ = 128.0      # fp8 scale on z8 (logit psum = AX*AZ*lg)
AP = 8192.0     # fp8 scale on Pn

F32 = mybir.dt.float32
BF16 = mybir.dt.bfloat16
F8 = mybir.dt.float8e4
AF = mybir.ActivationFunctionType
ALU = mybir.AluOpType
DR = mybir.MatmulPerfMode.DoubleRow


def _bcast(ap_row, parts):
    """Partition-broadcast a [1, n] DRAM AP to [parts, n] (step-0 partition)."""
    return bass.AP(
        tensor=ap_row.tensor,
        offset=ap_row.offset,
        ap=[[0, parts], list(ap_row.ap[-1])],
    )


def build_nc(t=T, d=D, h=H, e=E):
    """Build the per-core SPMD program. Returns a compiled Bacc."""
    TC = t // 512   # t chunks (512-wide psum free dim)
    SB = t // 128   # s blocks
    DC = d // 128   # d chunks (contraction for projections)
    DB = d // 128   # d blocks (z free dim / transposed-output partitions)

    sc = 1.0 / (AX * AZ)

    nc = bacc.Bacc("TRN2", target_bir_lowering=False, debug=False)

    # qt holds AX * x^T partition-major (the AX is divided back out of
    # the G/M operands host-side)
    qt_d = nc.dram_tensor("qt", [128, DC, t], BF16, kind="ExternalInput").ap()
    gz_d = nc.dram_tensor("gz", [h, 128, DC, d], BF16, kind="ExternalInput").ap()
    mt_d = nc.dram_tensor("mt", [h, 128, DC, d], BF16, kind="ExternalInput").ap()
    w1_d = nc.dram_tensor("w1s", [128, h, DB], F32, kind="ExternalInput").ap()
    cs_d = nc.dram_tensor("cs", [h, d], F32, kind="ExternalInput").ap()
    boc_d = nc.dram_tensor("boc", [128, DB], F32, kind="ExternalInput").ap()
    out_d = nc.dram_tensor("out", [d, t], F32, kind="ExternalOutput").ap()

    with tile.TileContext(nc) as tc, ExitStack() as ctx:
        consts = ctx.enter_context(tc.tile_pool(name="consts", bufs=1))
        wpool = ctx.enter_context(tc.tile_pool(name="wpool", bufs=2))
        hpool = ctx.enter_context(tc.tile_pool(name="hpool", bufs=2))
        qkpool = ctx.enter_context(tc.tile_pool(name="qkpool", bufs=2))
        epool = ctx.enter_context(tc.tile_pool(name="epool", bufs=3))
        spool = ctx.enter_context(tc.tile_pool(name="spool", bufs=2))
        at_pool = ctx.enter_context(tc.tile_pool(name="at_pool", bufs=3, space="PSUM"))
        mm_pool = ctx.enter_context(tc.tile_pool(name="mm_pool", bufs=5, space="PSUM"))

        # ---- persistent loads -------------------------------------------
        qt_sb = consts.tile([128, DC, t], BF16)
        nc.sync.dma_start(out=qt_sb[:, 0, :], in_=qt_d[:, 0, :])
        w1_sb = consts.tile([128, h, DB], F32)
        nc.sync.dma_start(out=w1_sb, in_=w1_d)
        boc_sb = consts.tile([128, DB], F32)
        nc.sync.dma_start(out=boc_sb, in_=boc_d)
        out_acc = consts.tile([128, DB, t], F32)
        uacc = consts.tile([128, DB], F32)
        ones_bf = consts.tile([128, 1], BF16)
        nc.vector.memset(ones_bf, 1.0)
        out_r = out_d.rearrange("(db p) t -> p db t", p=128)

        # ---- PE warm-up: dummy matmuls during the initial DMA wait ------
        scratch = consts.tile([128, 640], BF16)
        nc.vector.memset(scratch, 0.0)
        ps_w = mm_pool.tile([128, 512], F32, tag="mm")
        for _ in range(6):
            nc.tensor.matmul(
                ps_w, scratch[:, :128], scratch[:, 128:640], start=True, stop=True
            )

        gz_sb = [None] * h
        mt_sb = [None] * h
        c_bc = [None] * h
        zT8 = [None] * h
        P32 = [None] * h

        def load_head(hh, gate_mm=None):
            gz_sb[hh] = wpool.tile([128, DC, d], BF16, tag="gz", name="gz_sb")
            nc.sync.dma_start(out=gz_sb[hh], in_=gz_d[hh])
            mt_sb[hh] = wpool.tile([128, DC, d], BF16, tag="mt", name="mt_sb")
            mm = nc.sync.dma_start(out=mt_sb[hh], in_=mt_d[hh])
            c_bc[hh] = wpool.tile([128, d], F32, tag="c", name="c_bc")
            cc = nc.gpsimd.dma_start(
                out=c_bc[hh], in_=_bcast(cs_d[hh][None, :], 128)
            )
            if gate_mm is not None:
                for g in (mm, cc):
                    add_dep_helper(
                        g.ins, gate_mm.ins, reason="defer bulk load past cold start"
                    )

        def z_proj(hh):
            """z = x@(s*G) + w1 (bf16 matmul), cast to fp8 [d', t] tile."""
            zT8[hh] = qkpool.tile([128, DB, t], F8, tag="zT", name="zT8")
            first_mm = None
            for db in range(DB):
                for tch in range(TC):
                    tsl = slice(tch * 512, (tch + 1) * 512)
                    ps_z = mm_pool.tile([128, 512], F32, tag="mm")
                    for dc in range(DC):
                        mm = nc.tensor.matmul(
                            ps_z,
                            gz_sb[hh][:, dc, db * 128 : (db + 1) * 128],
                            qt_sb[:, dc, tsl],
                            start=(dc == 0),
                            stop=(dc == DC - 1),
                        )
                        if first_mm is None:
                            first_mm = mm
                    nc.vector.tensor_scalar_add(
                        zT8[hh][:, db, tsl], ps_z, w1_sb[:, hh, db : db + 1]
                    )
            return first_mm

        def p_proj(hh):
            """P32 = x @ M^T + c (bf16 matmul -> fp32 SBUF)."""
            P32[hh] = hpool.tile([128, SB, d], F32, tag="P32", name="P32")
            for sb in range(SB):
                ssl = slice(sb * 128, (sb + 1) * 128)
                pp = mm_pool.tile([128, 512], F32, tag="mm")
                for dc in range(DC):
                    nc.tensor.matmul(
                        pp[:, :d],
                        qt_sb[:, dc, ssl],
                        mt_sb[hh][:, dc, :],
                        start=(dc == 0),
                        stop=(dc == DC - 1),
                    )
                nc.vector.tensor_add(P32[hh][:, sb, :], pp[:, :d], c_bc[hh])

        pnb_pending = []

        def flush_u():
            """rank-1 term u[d] = sum_s Pnb[s,d] via bf16 N=1 matmuls."""
            while pnb_pending:
                uh, Pnb = pnb_pending.pop(0)
                for dt in range(DB):
                    dsl = slice(dt * 128, (dt + 1) * 128)
                    up = mm_pool.tile([128, 512], F32, tag="mm")
                    for sb in range(SB):
                        nc.tensor.matmul(
                            up[:, :1],
                            Pnb[:, sb, dsl],
                            ones_bf,
                            start=(sb == 0),
                            stop=(sb == SB - 1),
                        )
                    if uh == 0:
                        nc.scalar.activation(
                            uacc[:, dt : dt + 1], up[:, :1], AF.Copy
                        )
                    else:
                        nc.vector.tensor_add(
                            uacc[:, dt : dt + 1], uacc[:, dt : dt + 1], up[:, :1]
                        )

        # ---- prologue: head 0 z/P projections + the shared x8 cast ------
        load_head(0)
        nc.sync.dma_start(out=qt_sb[:, 1, :], in_=qt_d[:, 1, :])
        first_mm0 = z_proj(0)
        x8 = consts.tile([128, DC, t], F8)
        for dc in range(DC):
            nc.vector.tensor_scalar_mul(x8[:, dc, :], qt_sb[:, dc, :], 1.0)
        load_head_gate = first_mm0
        p_proj(0)

        for hh in range(h):
            if hh + 1 < h:
                load_head(hh + 1, gate_mm=load_head_gate if hh == 0 else None)

            # ---- scores (fp8 DR) -> exp -> R8 = exp-1 (+accum l) --------
            R8 = hpool.tile([128, SB, t], F8)
            lsum = spool.tile([128, SB], F32)
            for sb in range(SB):
                ssl = slice(sb * 128, (sb + 1) * 128)
                et = epool.tile([128, t], BF16)
                for tch in range(TC):
                    tsl = slice(tch * 512, (tch + 1) * 512)
                    at = at_pool.tile([128, 512], F32, tag="at")
                    nc.tensor.matmul(
                        at,
                        zT8[hh][:, :, ssl],
                        x8[:, :, tsl],
                        start=True,
                        stop=True,
                        perf_mode=DR,
                    )
                    nc.scalar.activation(et[:, tsl], at, AF.Exp, scale=sc)
                nc.vector.tensor_scalar(
                    R8[:, sb, :],
                    et,
                    1.0,
                    0.0,
                    op0=ALU.subtract,
                    op1=ALU.add,
                    accum_out=lsum[:, sb : sb + 1],
                )

            # ---- softmax denominators: rr2 = AP / l ---------------------
            lsS = spool.tile([128, SB], F32)
            rr2 = spool.tile([128, SB], F32)
            nc.vector.tensor_scalar(
                lsS, lsum, float(t), 1.0 / AP, op0=ALU.add, op1=ALU.mult
            )
            nc.vector.reciprocal(rr2, lsS)

            # ---- PE filler: next head's z projection --------------------
            if hh + 1 < h:
                z_proj(hh + 1)

            # ---- PE filler: previous head's u matmuls -------------------
            flush_u()

            # ---- Pn8 (fp8 for AV) on DVE --------------------------------
            Pn8 = hpool.tile([128, SB, d], F8)
            for sb in range(SB):
                nc.vector.tensor_scalar_mul(
                    Pn8[:, sb, :], P32[hh][:, sb, :], rr2[:, sb : sb + 1]
                )

            # ---- PE filler: next head's P projection --------------------
            if hh + 1 < h:
                p_proj(hh + 1)

            # ---- AV (fp8 DR): out^T[d,t] += Pn8^T-pairs x R8 ------------
            for dt in range(DB):
                dsl = slice(dt * 128, (dt + 1) * 128)
                for tch in range(TC):
                    tsl = slice(tch * 512, (tch + 1) * 512)
                    ot = at_pool.tile([128, 512], F32, tag="at")
                    for i in range(SB // 2):
                        nc.tensor.matmul(
                            ot,
                            Pn8[:, 2 * i : 2 * i + 2, dsl],
                            R8[:, 2 * i : 2 * i + 2, tsl],
                            start=(i == 0),
                            stop=(i == SB // 2 - 1),
                            perf_mode=DR,
                        )
                    if hh == 0:
                        nc.scalar.activation(out_acc[:, dt, tsl], ot, AF.Copy)
                    else:
                        nc.vector.tensor_add(
                            out_acc[:, dt, tsl], out_acc[:, dt, tsl], ot
                        )

            # ---- Pnb (bf16 for the rank-1 u term) on ScalarE ------------
            Pnb = hpool.tile([128, SB, d], BF16)
            for sb in range(SB):
                nc.scalar.mul(Pnb[:, sb, :], P32[hh][:, sb, :], rr2[:, sb : sb + 1])
            pnb_pending.append((hh, Pnb))

        flush_u()

        # ---- final: out = (out_acc + uacc + AP*bo) / AP, store ----------
        bvec = spool.tile([128, DB], F32)
        nc.vector.tensor_add(bvec, uacc, boc_sb)
        for dt in range(DB):
            nc.vector.tensor_scalar(
                out_acc[:, dt, :],
                out_acc[:, dt, :],
                bvec[:, dt : dt + 1],
                1.0 / AP,
                op0=ALU.add,
                op1=ALU.mult,
            )
            nc.sync.dma_start(out=out_r[:, dt, :], in_=out_acc[:, dt, :])

    nc.compile()
    return nc


_NC_CACHE = {}


def _get_nc(shape_key):
    if shape_key not in _NC_CACHE:
        _NC_CACHE[shape_key] = build_nc(*shape_key)
    return _NC_CACHE[shape_key]


def _pmajor(a, last):
    """[..., C*128, last] -> [..., 128, C, last] partition-major layout."""
    lead = a.shape[:-2]
    c = a.shape[-2] // 128
    return np.ascontiguousarray(
        a.reshape(*lead, c, 128, last).swapaxes(-3, -2)
    )


def _prep_inputs(Q, Wq, bq, Wk, bk, Wv, bv, Wo, bo):
    t, b, d = Q.shape
    h, e, _ = Wq.shape
    s = np.float32(1.0 / np.sqrt(e))
    bf = ml_dtypes.bfloat16
    Q = np.asarray(Q, np.float32)
    Wq = np.asarray(Wq, np.float32)
    Wk = np.asarray(Wk, np.float32)
    Wv = np.asarray(Wv, np.float32)
    Wo = np.asarray(Wo, np.float32)
    bk = np.asarray(bk, np.float32)
    bv = np.asarray(bv, np.float32)
    bo = np.asarray(bo, np.float32)
    # [B, 128, DC, T] partition-major AX * x^T per batch
    qt_all = _pmajor((Q * AX).transpose(1, 2, 0).astype(bf), t)
    # z side: Gz[d, d'] = (AZ*s/AX) * (Wk^T @ Wq)[d, d'],  w1 = AZ*s*Wq^T@bk
    gzs = np.stack([(AZ * s / AX) * (Wk[hh].T @ Wq[hh]) for hh in range(h)])
    gz = _pmajor(gzs.astype(bf), d)
    w1 = np.stack([(AZ * s) * (Wq[hh].T @ bk[hh]) for hh in range(h)])
    # P side: M_h = Wo_h @ Wv_h; mt stores M_h^T/AX partition-major over d'
    Wo_heads = Wo.reshape(d, h, e)
    mts = np.stack([(Wo_heads[:, hh, :] @ Wv[hh]).T / AX for hh in range(h)])
    mt = _pmajor(mts.astype(bf), d)
    cs = np.stack([bv[hh] @ Wo_heads[:, hh, :].T for hh in range(h)])
    shared = {
        "gz": gz,
        "mt": mt,
        "w1s": np.ascontiguousarray(w1.reshape(h, -1, 128).transpose(2, 0, 1)),
        "cs": np.ascontiguousarray(cs.astype(np.float32)),
        "boc": np.ascontiguousarray((bo * AP).reshape(-1, 128).T.astype(np.float32)),
    }
    in_maps = [
        {"qt": np.ascontiguousarray(qt_all[bb]), **shared} for bb in range(b)
    ]
    return in_maps, (t, d, h, e)


def kernel(Q, Wq, bq, Wk, bk, Wv, bv, Wo, bo, _trace=False):
    in_maps, (t, d, h, e) = _prep_inputs(Q, Wq, bq, Wk, bk, Wv, bv, Wo, bo)
    nc = _get_nc((t, d, h, e))
    res = bass_utils.run_bass_kernel_spmd(
        nc, in_maps, core_ids=list(range(len(in_maps))), trace=_trace
    )
    # per-core output is out^T [D, T]; transpose back and stack over batch
    out = np.stack(
        [res.results[bb]["out"].T for bb in range(len(in_maps))], axis=1
    )
    if _trace:
        kernel.last_results = res
    return np.ascontiguousarray(out.astype(np.float32))


# revision 15
# speedup vs baseline: 4.4678x; 1.0648x over previous
"""Multi-head attention (softmax over the QUERY axis) on 8 TRN2 NeuronCores.

Problem shapes: Q [T=1024, B=8, D=256]; per-head projections Wq/Wk/Wv
[H=8, E=512, D=256]; Wo [D=256, H*E=4096]. Data-parallel over batch B.

Two exact algebraic restructurings (both exploit E > D):

1. V/output side: since o_h = attn_h @ v_h and v_h = x@Wv_h^T + bv_h,
       out = sum_h attn_h @ (x @ M_h^T + c_h) + bo,
       M_h = Wo_h @ Wv_h  (D x D, host),  c_h = bv_h @ Wo_h^T.
   Removes the V projection, the E-wide attn@V matmul and the output
   projection.

2. Q/K side: q_t . k_s = x_t . (G_h x_s) with G_h = Wq_h^T @ Wk_h
   (D x D, host).  The softmax is over the QUERY axis t, so per-key
   additive terms (bq.k_s, bq.bk) cancel EXACTLY and only
   w1_h = s*Wq_h^T @ bk_h survives as a bias on the z projection:
       lg[t,s] = x_t . z_s,   z = x @ (s*G_h) + w1_h.
   Removes both the q and k projections; scores contract over D=256
   instead of E=512.

Per-head MACs: 1611M -> 670M.  The scores matmul runs in fp8 (e4m3)
DoubleRow (z8 x x8, both cast with power-of-2 scales); the attention
output Pn^T x Ex runs in bf16 (Ex = exp from the ScalarE with the
softmax denominators l[s] from its accum_out; Pn = (x@M^T + c)*AP/l).

The head loop is software-pipelined two-deep: head h's scores matmuls
are interleaved with head h-1's AV matmuls and head h+1's z/P
projection matmuls, so the PE never head-of-line blocks on the ACT exp
pace and never idles while the exp -> l -> rr -> Pn chain drains
(PE-idle gaps >3.4us re-throttle the HAM clock gate to half rate).
"""

import sys

sys.path.insert(0, "/opt/trn_rl_repo")

from contextlib import ExitStack

import ml_dtypes
import numpy as np

import concourse.bass as bass
import concourse.tile as tile
from concourse.tile import add_dep_helper
from concourse import bacc, bass_utils, mybir

T, B, D, H, E = 1024, 8, 256, 8, 512
N_CORES = 8
AX = 8.0        # fp8 scale on x8 (folded into qt host-side)
AZ = 128.0      # fp8 scale on z8 (logit psum = AX*AZ*lg)
AP = 8192.0     # scale on Pn / out_acc

F32 = mybir.dt.float32
BF16 = mybir.dt.bfloat16
F8 = mybir.dt.float8e4
AF = mybir.ActivationFunctionType
ALU = mybir.AluOpType
DR = mybir.MatmulPerfMode.DoubleRow


def _bcast(ap_row, parts):
    """Partition-broadcast a [1, n] DRAM AP to [parts, n] (step-0 partition)."""
    return bass.AP(
        tensor=ap_row.tensor,
        offset=ap_row.offset,
        ap=[[0, parts], list(ap_row.ap[-1])],
    )


def build_nc(t=T, d=D, h=H, e=E):
    """Build the per-core SPMD program. Returns a compiled Bacc."""
    TC = t // 512   # t chunks (512-wide psum free dim)
    SB = t // 128   # s blocks
    DC = d // 128   # d chunks (contraction for projections)
    DB = d // 128   # d blocks (z free dim / transposed-output partitions)

    sc = 1.0 / (AX * AZ)

    nc = bacc.Bacc("TRN2", target_bir_lowering=False, debug=False)

    qt_d = nc.dram_tensor("qt", [128, DC, t], BF16, kind="ExternalInput").ap()
    gz_d = nc.dram_tensor("gz", [h, 128, DC, d], BF16, kind="ExternalInput").ap()
    mt_d = nc.dram_tensor("mt", [h, 128, DC, d], BF16, kind="ExternalInput").ap()
    w1_d = nc.dram_tensor("w1s", [128, h, DB], F32, kind="ExternalInput").ap()
    cs_d = nc.dram_tensor("cs", [h, d], F32, kind="ExternalInput").ap()
    boc_d = nc.dram_tensor("boc", [128, DB], F32, kind="ExternalInput").ap()
    out_d = nc.dram_tensor("out", [d, t], F32, kind="ExternalOutput").ap()

    with tile.TileContext(nc) as tc, ExitStack() as ctx:
        consts = ctx.enter_context(tc.tile_pool(name="consts", bufs=1))
        wpool = ctx.enter_context(tc.tile_pool(name="wpool", bufs=2))
        hpool = ctx.enter_context(tc.tile_pool(name="hpool", bufs=2))
        qkpool = ctx.enter_context(tc.tile_pool(name="qkpool", bufs=2))
        spool = ctx.enter_context(tc.tile_pool(name="spool", bufs=2))
        at_pool = ctx.enter_context(tc.tile_pool(name="at_pool", bufs=3, space="PSUM"))
        mm_pool = ctx.enter_context(tc.tile_pool(name="mm_pool", bufs=5, space="PSUM"))

        # ---- persistent loads -------------------------------------------
        qt_sb = consts.tile([128, DC, t], BF16)
        nc.sync.dma_start(out=qt_sb[:, 0, :], in_=qt_d[:, 0, :])
        w1_sb = consts.tile([128, h, DB], F32)
        nc.sync.dma_start(out=w1_sb, in_=w1_d)
        boc_sb = consts.tile([128, DB], F32)
        nc.sync.dma_start(out=boc_sb, in_=boc_d)
        out_acc = consts.tile([128, DB, t], F32)
        out_r = out_d.rearrange("(db p) t -> p db t", p=128)

        # ---- PE warm-up: dummy matmuls during the initial DMA wait ------
        scratch = consts.tile([128, 640], BF16)
        nc.vector.memset(scratch, 0.0)
        ps_w = mm_pool.tile([128, 512], F32, tag="mm")
        for _ in range(6):
            nc.tensor.matmul(
                ps_w, scratch[:, :128], scratch[:, 128:640], start=True, stop=True
            )

        gz_sb = [None] * h
        mt_sb = [None] * h
        c_bc = [None] * h
        zT8 = [None] * h
        P32 = [None] * h
        Pnb = [None] * h
        Ex = [None] * h

        def load_head(hh, gate_mm=None):
            gz_sb[hh] = wpool.tile([128, DC, d], BF16, tag="gz", name="gz_sb")
            nc.sync.dma_start(out=gz_sb[hh], in_=gz_d[hh])
            mt_sb[hh] = wpool.tile([128, DC, d], BF16, tag="mt", name="mt_sb")
            mm = nc.sync.dma_start(out=mt_sb[hh], in_=mt_d[hh])
            c_bc[hh] = wpool.tile([128, d], F32, tag="c", name="c_bc")
            cc = nc.gpsimd.dma_start(
                out=c_bc[hh], in_=_bcast(cs_d[hh][None, :], 128)
            )
            if gate_mm is not None:
                for g in (mm, cc):
                    add_dep_helper(
                        g.ins, gate_mm.ins, reason="defer bulk load past cold start"
                    )

        def z_group(hh, db, tch):
            """One psum-group of the z projection: z = x@(s*G) + w1 -> fp8."""
            tsl = slice(tch * 512, (tch + 1) * 512)
            ps_z = mm_pool.tile([128, 512], F32, tag="mm")
            first = None
            for dc in range(DC):
                mm = nc.tensor.matmul(
                    ps_z,
                    gz_sb[hh][:, dc, db * 128 : (db + 1) * 128],
                    qt_sb[:, dc, tsl],
                    start=(dc == 0),
                    stop=(dc == DC - 1),
                )
                first = first or mm
            nc.vector.tensor_scalar_add(
                zT8[hh][:, db, tsl], ps_z, w1_sb[:, hh, db : db + 1]
            )
            return first

        def p_group(hh, sb):
            """One psum-group of the P projection: P32 = x @ M^T + c."""
            ssl = slice(sb * 128, (sb + 1) * 128)
            pp = mm_pool.tile([128, 512], F32, tag="mm")
            for dc in range(DC):
                nc.tensor.matmul(
                    pp[:, :d],
                    qt_sb[:, dc, ssl],
                    mt_sb[hh][:, dc, :],
                    start=(dc == 0),
                    stop=(dc == DC - 1),
                )
            nc.vector.tensor_add(P32[hh][:, sb, :], pp[:, :d], c_bc[hh])

        def av_group(hh, dt, tch):
            """One psum-group of the AV matmul (bf16): out^T += Pn^T x Ex."""
            dsl = slice(dt * 128, (dt + 1) * 128)
            tsl = slice(tch * 512, (tch + 1) * 512)
            ot = mm_pool.tile([128, 512], F32, tag="mm")
            for sb in range(SB):
                nc.tensor.matmul(
                    ot,
                    Pnb[hh][:, sb, dsl],
                    Ex[hh][:, sb, tsl],
                    start=(sb == 0),
                    stop=(sb == SB - 1),
                )
            if hh == 0:
                nc.scalar.activation(out_acc[:, dt, tsl], ot, AF.Copy)
            else:
                nc.vector.tensor_add(out_acc[:, dt, tsl], out_acc[:, dt, tsl], ot)

        # ---- prologue: head 0 z/P projections + the shared x8 cast ------
        load_head(0)
        nc.sync.dma_start(out=qt_sb[:, 1, :], in_=qt_d[:, 1, :])
        zT8[0] = qkpool.tile([128, DB, t], F8, tag="zT", name="zT8")
        first_mm0 = None
        for db in range(DB):
            for tch in range(TC):
                mm = z_group(0, db, tch)
                first_mm0 = first_mm0 or mm
        x8 = consts.tile([128, DC, t], F8)
        for dc in range(DC):
            nc.vector.tensor_scalar_mul(x8[:, dc, :], qt_sb[:, dc, :], 1.0)
        P32[0] = hpool.tile([128, SB, d], F32, tag="P32", name="P32")
        for sb in range(SB):
            p_group(0, sb)

        for hh in range(h):
            if hh + 1 < h:
                load_head(hh + 1, gate_mm=first_mm0 if hh == 0 else None)
                zT8[hh + 1] = qkpool.tile([128, DB, t], F8, tag="zT", name="zT8")
                P32[hh + 1] = hpool.tile([128, SB, d], F32, tag="P32", name="P32")

            # filler matmul groups to interleave with this head's scores:
            # previous head's AV + next head's z/P projections
            fillers = []
            if hh > 0:
                for dt in range(DB):
                    for tch in range(TC):
                        fillers.append(lambda dt=dt, tch=tch: av_group(hh - 1, dt, tch))
            if hh + 1 < h:
                for db in range(DB):
                    for tch in range(TC):
                        fillers.append(lambda db=db, tch=tch: z_group(hh + 1, db, tch))
                for sb in range(SB):
                    fillers.append(lambda sb=sb: p_group(hh + 1, sb))

            # ---- scores (fp8 DR) -> exp on ACT (+accum l), interleaved --
            Ex[hh] = hpool.tile([128, SB, t], BF16, tag="Ex", name="Ex")
            lsum2 = spool.tile([128, SB, TC], F32)
            fi = 0
            for sb in range(SB):
                ssl = slice(sb * 128, (sb + 1) * 128)
                for tch in range(TC):
                    tsl = slice(tch * 512, (tch + 1) * 512)
                    at = at_pool.tile([128, 512], F32, tag="at")
                    nc.tensor.matmul(
                        at,
                        zT8[hh][:, :, ssl],
                        x8[:, :, tsl],
                        start=True,
                        stop=True,
                        perf_mode=DR,
                    )
                    nc.scalar.activation(
                        Ex[hh][:, sb, tsl],
                        at,
                        AF.Exp,
                        scale=sc,
                        accum_out=lsum2[:, sb, tch : tch + 1],
                    )
                for _ in range(2):
                    if fi < len(fillers):
                        fillers[fi]()
                        fi += 1
            while fi < len(fillers):
                fillers[fi]()
                fi += 1

            # ---- softmax denominators: rr2 = AP / l ---------------------
            ls = spool.tile([128, SB], F32)
            lsS = spool.tile([128, SB], F32)
            rr2 = spool.tile([128, SB], F32)
            nc.vector.tensor_add(ls, lsum2[:, :, 0], lsum2[:, :, 1])
            nc.vector.tensor_scalar_mul(lsS, ls, 1.0 / AP)
            nc.vector.reciprocal(rr2, lsS)

            # ---- Pnb (bf16 stationary operand of AV) on DVE -------------
            Pnb[hh] = hpool.tile([128, SB, d], BF16, tag="Pnb", name="Pnb")
            for sb in range(SB):
                nc.vector.tensor_scalar_mul(
                    Pnb[hh][:, sb, :], P32[hh][:, sb, :], rr2[:, sb : sb + 1]
                )

        # ---- epilogue: last head's AV -----------------------------------
        for dt in range(DB):
            for tch in range(TC):
                av_group(h - 1, dt, tch)

        # ---- final: out = (out_acc + AP*bo) / AP, store -----------------
        for dt in range(DB):
            nc.vector.tensor_scalar(
                out_acc[:, dt, :],
                out_acc[:, dt, :],
                boc_sb[:, dt : dt + 1],
                1.0 / AP,
                op0=ALU.add,
                op1=ALU.mult,
            )
            nc.sync.dma_start(out=out_r[:, dt, :], in_=out_acc[:, dt, :])

    nc.compile()
    return nc


_NC_CACHE = {}


def _get_nc(shape_key):
    if shape_key not in _NC_CACHE:
        _NC_CACHE[shape_key] = build_nc(*shape_key)
    return _NC_CACHE[shape_key]


def _pmajor(a, last):
    """[..., C*128, last] -> [..., 128, C, last] partition-major layout."""
    lead = a.shape[:-2]
    c = a.shape[-2] // 128
    return np.ascontiguousarray(
        a.reshape(*lead, c, 128, last).swapaxes(-3, -2)
    )


def _prep_inputs(Q, Wq, bq, Wk, bk, Wv, bv, Wo, bo):
    t, b, d = Q.shape
    h, e, _ = Wq.shape
    s = np.float32(1.0 / np.sqrt(e))
    bf = ml_dtypes.bfloat16
    Q = np.asarray(Q, np.float32)
    Wq = np.asarray(Wq, np.float32)
    Wk = np.asarray(Wk, np.float32)
    Wv = np.asarray(Wv, np.float32)
    Wo = np.asarray(Wo, np.float32)
    bk = np.asarray(bk, np.float32)
    bv = np.asarray(bv, np.float32)
    bo = np.asarray(bo, np.float32)
    # [B, 128, DC, T] partition-major AX * x^T per batch
    qt_all = _pmajor((Q * AX).transpose(1, 2, 0).astype(bf), t)
    # z side: Gz = (AZ*s/AX) * (Wk^T @ Wq),  w1 = AZ*s*Wq^T@bk
    gzs = np.stack([(AZ * s / AX) * (Wk[hh].T @ Wq[hh]) for hh in range(h)])
    gz = _pmajor(gzs.astype(bf), d)
    w1 = np.stack([(AZ * s) * (Wq[hh].T @ bk[hh]) for hh in range(h)])
    # P side: M_h = Wo_h @ Wv_h; mt stores M_h^T/AX partition-major over d'
    Wo_heads = Wo.reshape(d, h, e)
    mts = np.stack([(Wo_heads[:, hh, :] @ Wv[hh]).T / AX for hh in range(h)])
    mt = _pmajor(mts.astype(bf), d)
    cs = np.stack([bv[hh] @ Wo_heads[:, hh, :].T for hh in range(h)])
    shared = {
        "gz": gz,
        "mt": mt,
        "w1s": np.ascontiguousarray(w1.reshape(h, -1, 128).transpose(2, 0, 1)),
        "cs": np.ascontiguousarray(cs.astype(np.float32)),
        "boc": np.ascontiguousarray((bo * AP).reshape(-1, 128).T.astype(np.float32)),
    }
    in_maps = [
        {"qt": np.ascontiguousarray(qt_all[bb]), **shared} for bb in range(b)
    ]
    return in_maps, (t, d, h, e)


def kernel(Q, Wq, bq, Wk, bk, Wv, bv, Wo, bo, _trace=False):
    in_maps, (t, d, h, e) = _prep_inputs(Q, Wq, bq, Wk, bk, Wv, bv, Wo, bo)
    nc = _get_nc((t, d, h, e))
    res = bass_utils.run_bass_kernel_spmd(
        nc, in_maps, core_ids=list(range(len(in_maps))), trace=_trace
    )
    # per-core output is out^T [D, T]; transpose back and stack over batch
    out = np.stack(
        [res.results[bb]["out"].T for bb in range(len(in_maps))], axis=1
    )
    if _trace:
        kernel.last_results = res
    return np.ascontiguousarray(out.astype(np.float32))
